# revision 47
# baseline (speedup 1.0000x reference)
"""HGCN decoder on 8 trn2 NeuronCores.

Strategy: nodes are sorted by in-degree, grouped into 128-node tiles, and the
tiles are dealt round-robin across the 8 cores (graph-parallel by destination
node).  Each core:
  - runs the node-wise hyperbolic math on its 4096 nodes,
  - publishes its tangent-space table shard, AllGathers the full [32768, 64]
    table to DRAM,
  - aggregates messages with `dma_gather` (padded per-tile CSR) followed by a
    strided reduce on the vector engine,
  - finishes with the euclidean readout matmul.

Key algebraic simplifications vs the literal reference (all exact up to fp
rounding, validated against the jax reference):
  - proj-then-logmap0 collapses: artanh(min(tanh(r), 1-eps)) == min(r, R*)
    with R* = artanh(1-eps).  This removes every Ln/artanh evaluation and all
    but one final Tanh, so the scalar chains are pure min/mult/recip/sqrt.
  - positive per-node scales commute through relu and matmuls, so the vector
    state is kept UNSCALED (raw) and only combined per-node scalars are
    carried between stages.  The only materialized scalings are the published
    message table (xt = mv_raw * sigma) and the final readout.
Activation-table note: the whole kernel needs only {Square, Sqrt, Relu, Copy}
(one table) plus a single trailing Tanh (second table) => 2 table loads total.

All graph preprocessing happens host-side in numpy; the device only sees
dense tables.
"""

import numpy as np

N = 32768
E = 1015808
D = 64
C = 8          # cores
NL = N // C    # 4096 nodes per core
P = 128        # partitions / tile
T = NL // P    # 32 tiles per core
Q = 8          # tiles per chain-quarter
NQ = T // Q
MAXN = 1.0 - 4e-3   # PROJ_EPS boundary for c=1
EPS = 1e-15
MAX_TANH = 15.0
R_STAR = float(np.arctanh(np.float64(1.0 - 4e-3)))  # artanh(MAXN) = 3.10635...


def _build_tables(rows, cols, edge_mask, node_mask):
    """Permute nodes by degree, deal tiles round-robin to cores, and build the
    per-core padded gather tables (int16 indices wrapped the way
    InstDMAGatherAnt wants them) plus matching weight tables."""
    deg = np.bincount(rows, minlength=N)
    order = np.argsort(-deg, kind="stable")
    # global tile j -> core j%C, sorted-slot j//C; slots are then re-ordered so
    # the smallest-K slot runs FIRST (cheap desc-gen before the pipeline
    # fills) and the second-smallest LAST (short exposed tail):
    #   slot order = [smallest, biggest, ..., 2nd smallest]
    S = np.empty(T, dtype=np.int64)
    S[0] = T - 1
    S[1:] = np.arange(T - 1)
    Sinv = np.empty(T, dtype=np.int64)
    Sinv[S] = np.arange(T)
    perm = np.empty(N, dtype=np.int64)
    j = np.arange(N) // P                     # global tile of sorted rank r
    c = j % C
    t = Sinv[j // C]
    p = np.arange(N) % P
    perm[c * NL + t * P + p] = order          # perm[g] = original node id
    pos = np.empty(N, dtype=np.int64)
    pos[perm] = np.arange(N)

    # gather-table row id for permuted position g=(c,t,p):
    #   AllGather concatenates per-core [P, T*D] blocks, so
    #   row_id = c*NL + p*T + t
    gg = np.arange(N)
    gc, gr = gg // NL, gg % NL
    gt, gp_ = gr // P, gr % P
    rowid = gc * NL + gp_ * T + gt            # [g] -> table row
    dstpos = pos[rows]
    eorder = np.argsort(dstpos, kind="stable")
    src_sorted = rowid[pos[cols[eorder]]]     # gather table rows, 0..N-1
    w_sorted = edge_mask[eorder, 0].astype(np.float64)
    cnts = np.bincount(dstpos, minlength=N)
    offs = np.zeros(N + 1, dtype=np.int64)
    np.cumsum(cnts, out=offs[1:])

    # per-slot K: max count over the 8 cores' tiles in that slot
    cnts_g = cnts.reshape(C, T, P)
    Ks = np.maximum(cnts_g.max(axis=(0, 2)), 1).astype(np.int64)   # [T]

    IDXC = int(8 * Ks.sum())
    WTC = int(Ks.sum())
    idx_dev = np.zeros((C, P, IDXC), np.int16)
    wt_dev = np.zeros((C, P, WTC), np.float32)
    nm = node_mask[:, 0].astype(np.float64)
    ioff = woff = 0
    ar = None
    for t in range(T):
        K = int(Ks[t])
        if ar is None or ar.shape[1] != K:
            ar = np.arange(K)[None, :]
        for cc in range(C):
            base = cc * NL + t * P
            cn = cnts[base:base + P]
            take = offs[base:base + P][:, None] + ar          # [P, K]
            valid = ar < cn[:, None]
            take_c = np.minimum(take, E - 1)
            nb = np.where(valid, src_sorted[take_c], 0)
            wl = np.where(valid, w_sorted[take_c], 0.0)
            wl = wl * nm[perm[base:base + P]][:, None]
            il = nb.T.reshape(-1)                             # i = g*128+p
            ch = il.reshape(8 * K, 16).T                      # [16, 8K]
            idx_dev[cc, :, ioff:ioff + 8 * K] = np.tile(ch, (8, 1)).astype(np.int16)
            wt_dev[cc, :, woff:woff + K] = wl.astype(np.float32)
        ioff += 8 * K
        woff += K
    # pad counts per (core, slot, partition) for the pad-subtract path
    pc_dev = np.zeros((C, 1, T * P), np.float32)
    for t in range(T):
        K = int(Ks[t])
        for cc in range(C):
            base = cc * NL + t * P
            pc_dev[cc, 0, t * P:(t + 1) * P] = K - cnts[base:base + P]
    allones = bool(np.all(edge_mask == 1.0) and np.all(node_mask == 1.0))
    return perm, Ks, idx_dev, wt_dev, IDXC, WTC, pc_dev, allones


def _build_program(Ks, IDXC, WTC, use_wt=True, sim=False):
    import concourse.bacc as bacc
    import concourse.bass as bass
    import concourse.mybir as mybir
    import concourse.tile as tile
    from concourse import library_config
    from concourse.masks import make_identity

    f32 = mybir.dt.float32
    i16 = mybir.dt.int16
    AF = mybir.ActivationFunctionType
    OP = mybir.AluOpType
    X = mybir.AxisListType.X

    nc = bacc.Bacc("TRN2", target_bir_lowering=False, debug=False,
                   num_devices=1 if sim else C)

    h_in = nc.dram_tensor("h_in", [P, T * D], f32, kind="ExternalInput")
    idx_in = nc.dram_tensor("idx_in", [P, IDXC], i16, kind="ExternalInput")
    wt_in = nc.dram_tensor("wt_in", [P, WTC], f32, kind="ExternalInput")
    w0t_in = nc.dram_tensor("w0t_in", [2 * D, D], f32, kind="ExternalInput")
    w1t_in = nc.dram_tensor("w1t_in", [2 * D, D], f32, kind="ExternalInput")
    wot_in = nc.dram_tensor("wot_in", [2 * D, 16], f32, kind="ExternalInput")
    pc_in = nc.dram_tensor("pc_in", [1, T * P], f32, kind="ExternalInput")
    out_dram = nc.dram_tensor("out", [P, T * 16], f32, kind="ExternalOutput")
    xt_shard = nc.dram_tensor("xt_shard", [P, T * D], f32)
    xt_table = nc.dram_tensor("xt_table", [N, D], f32, addr_space="Shared")
    groups = [list(range(C))]

    with tile.TileContext(nc) as tc:
        nc.gpsimd.load_library(library_config.mlp)
        import contextlib
        ctx = contextlib.ExitStack()
        with ctx:
            const = ctx.enter_context(tc.tile_pool(name="const", bufs=1))
            sqp = ctx.enter_context(tc.tile_pool(name="sq", bufs=2))
            xtp = ctx.enter_context(tc.tile_pool(name="xtp", bufs=3))
            gp = ctx.enter_context(tc.tile_pool(name="gp", bufs=4))
            scp = ctx.enter_context(tc.tile_pool(name="scp", bufs=3))
            psT = ctx.enter_context(tc.tile_pool(name="psT", bufs=2, space="PSUM"))
            psM = ctx.enter_context(tc.tile_pool(name="psM", bufs=2, space="PSUM"))
            psC = ctx.enter_context(tc.tile_pool(name="psC", bufs=2, space="PSUM"))

            ident = const.tile([P, P], f32)
            make_identity(nc, ident[:])
            # weights duplicated across both partition halves so matmuls can
            # source lhsT from either half of a paired transpose
            w0t_sb = const.tile([2 * D, D], f32)
            w1t_sb = const.tile([2 * D, D], f32)
            wot_sb = const.tile([2 * D, 16], f32)
            idx_sb = const.tile([P, IDXC], i16)
            pc_sb = const.tile([1, T * P], f32)
            if use_wt:
                wt_sb = const.tile([P, WTC], f32)

            u_sb = const.tile([P, T * D], f32)      # raw vector state / agg
            mv_sb = const.tile([P, T * D], f32)     # W@u then scaled messages
            out_sb = const.tile([P, T * 16], f32)
            # long-lived per-node scalars, [P, T] column t = tile t
            A_sb = const.tile([P, T], f32)    # artanh(||x||) of current state
            rn_sb = const.tile([P, T], f32)   # 1 / ||u_raw||
            y_sb = const.tile([P, T], f32)    # A * rn (sigma chain operand)
            t2_sb = const.tile([P, T], f32)   # final tanh argument (layer 2)
            na2_sb = const.tile([P, T], f32)
            nu2_sb = const.tile([P, T], f32)
            mx2_sb = const.tile([P, T], f32)
            sg_sb = const.tile([P, T], f32)   # publish scale sigma per tile
            sf_sb = const.tile([P, T], f32)   # final readout scale

            # load order: h quarter 0 + W0 first (intro/linear need them), the
            # gather metadata afterwards so it streams under the intro compute
            QW = Q * D
            nc.sync.dma_start(out=u_sb[:, 0:QW], in_=h_in[:, 0:QW])
            nc.sync.dma_start(out=w0t_sb[:], in_=w0t_in[:])
            for q in range(1, NQ):
                nc.sync.dma_start(out=u_sb[:, q * QW:(q + 1) * QW],
                                  in_=h_in[:, q * QW:(q + 1) * QW])
            nc.sync.dma_start(out=w1t_sb[:], in_=w1t_in[:])
            nc.sync.dma_start(out=wot_sb[:], in_=wot_in[:])
            nc.sync.dma_start(out=pc_sb[:], in_=pc_in[:])
            nc.sync.dma_start(out=idx_sb[:], in_=idx_in[:])
            if use_wt:
                nc.sync.dma_start(out=wt_sb[:], in_=wt_in[:])

            def ts(t, w=D):
                return slice(t * w, (t + 1) * w)

            def act(out, in_, f, **kw):
                nc.scalar.activation(out, in_, f, **kw)

            def intro_quarter(q):
                """A = min(||h||, R*), rn = 1/||h||, y = A*rn for quarter q."""
                cols = ts(q, Q)
                n02 = scp.tile([P, Q], f32, tag="n02")
                sqb = sqp.tile([P, Q * D], f32, tag="sqb")
                act(sqb[:], u_sb[:, ts(q, Q * D)], AF.Square)
                nc.vector.tensor_reduce(
                    n02[:], sqb[:].rearrange("p (t d) -> p t d", d=D),
                    axis=X, op=OP.add)
                z = scp.tile([P, Q], f32, tag="z0")
                nc.vector.reciprocal(z[:], n02[:])
                act(rn_sb[:, cols], z[:], AF.Sqrt)          # 1/n0
                n0 = scp.tile([P, Q], f32, tag="n0")
                nc.vector.tensor_tensor(n0[:], n02[:], rn_sb[:, cols],
                                        op=OP.mult)         # n0
                nc.vector.tensor_scalar_min(A_sb[:, cols], n0[:], R_STAR)
                nc.vector.tensor_tensor(y_sb[:, cols], A_sb[:, cols],
                                        rn_sb[:, cols], op=OP.mult)

            def mm_tiles(tiles, w_sb, dst_sb, ow):
                """transpose+matmul u[tiles] @ W^T -> dst_sb (free width ow),
                batching up to 4 matmul outputs per PSUM tile/copy."""
                t0, t1 = tiles.start, tiles.stop
                t = t0
                while t < t1:
                    nq = min(4, t1 - t)
                    pm = psM.tile([P, 4 * ow], f32, tag="mv")
                    for j in range(nq):
                        tt = t + j
                        if j % 2 == 0:
                            np2 = min(2, t1 - tt)
                            ps = psT.tile([P, P], f32, tag="xT2")
                            nc.tensor.transpose(
                                out=ps[0:np2 * D, :],
                                in_=u_sb[:, tt * D:(tt + np2) * D],
                                identity=ident[:])
                            xT2 = xtp.tile([P, P], f32, tag="xT2sb")
                            nc.vector.tensor_copy(xT2[0:np2 * D, :],
                                                  ps[0:np2 * D, :])
                        h2 = j % 2
                        nc.tensor.matmul(out=pm[:, j * ow:(j + 1) * ow],
                                         lhsT=xT2[h2 * D:(h2 + 1) * D, :],
                                         rhs=w_sb[h2 * D:(h2 + 1) * D, :],
                                         start=True, stop=True)
                    act(dst_sb[:, t * ow:(t + nq) * ow], pm[:, :nq * ow],
                        AF.Copy)
                    t += nq

            def linear_block(w_sb, tiles, publish=True):
                """mv_raw = u @ W^T for `tiles`; sigma = min(y, R*/mraw);
                mv *= sigma; publish.  (y = A*rn precomputed per node.)"""
                t0, t1 = tiles.start, tiles.stop
                nt = t1 - t0
                mm_tiles(tiles, w_sb, mv_sb, D)
                # batched ||mv||^2 then the fused sigma chain
                for j in range((nt + Q - 1) // Q):
                    c0 = t0 + j * Q
                    c1 = min(c0 + Q, t1)
                    sqb = sqp.tile([P, Q * D], f32, tag="sqb")
                    act(sqb[:, :(c1 - c0) * D], mv_sb[:, c0 * D:c1 * D], AF.Square)
                    nc.vector.tensor_reduce(
                        mx2_sb[:, c0:c1],
                        sqb[:, :(c1 - c0) * D].rearrange("p (t d) -> p t d", d=D),
                        axis=X, op=OP.add)
                cols = slice(t0, t1)
                rm = scp.tile([P, T], f32, tag="rm")
                nc.vector.reciprocal(rm[:, cols], mx2_sb[:, cols])
                act(rm[:, cols], rm[:, cols], AF.Sqrt, scale=R_STAR * R_STAR)
                nc.vector.tensor_tensor(sg_sb[:, cols], y_sb[:, cols],
                                        rm[:, cols], op=OP.min)
                # mv *= sigma, broadcast over d in one strided multiply
                mv3 = mv_sb[:, t0 * D:t1 * D].rearrange("p (t d) -> p t d", d=D)
                sg_ap = sg_sb[:, cols]
                sgv = bass.AP(sg_ap.tensor, sg_ap.offset,
                              list(sg_ap.ap) + [[0, D]])
                nc.vector.tensor_tensor(mv3, mv3, sgv, op=OP.mult)
                if publish:
                    nc.sync.dma_start(out=xt_shard[:, t0 * D:t1 * D],
                                      in_=mv_sb[:, t0 * D:t1 * D])

            def readout_block(tiles):
                mm_tiles(tiles, wot_sb, out_sb, 16)

            def allgather(t0=0, t1=T):
                """AllGather the shard columns for tiles [t0, t1).  Table row
                of (core c, partition p, tile t) is (c*P+p)*T + t, so a tile
                range is a strided row block."""
                shard_part = xt_shard[:, t0 * D:t1 * D].rearrange(
                    "p (t d) -> p t d", d=D)
                if sim:
                    nc.sync.dma_start(
                        out=xt_table[0:NL, :].rearrange(
                            "(q t) d -> q t d", t=T)[:, t0:t1, :],
                        in_=shard_part)
                else:
                    nc.gpsimd.collective_compute(
                        "AllGather", mybir.AluOpType.bypass, replica_groups=groups,
                        ins=[shard_part],
                        outs=[xt_table[:, :].rearrange(
                            "(q t) d -> q t d", t=T)[:, t0:t1, :]])

            def post_agg_group(g0, g1, last_layer):
                """norms of agg & relu(agg); s' chain; t = nu*s'; A'/rn' (or
                stash t2 for the final tanh)."""
                w = g1 - g0
                cols = slice(g0, g1)
                csl = slice(g0 * D, g1 * D)
                sqb = sqp.tile([P, Q * D], f32, tag="sqb")
                act(sqb[:, :w * D], u_sb[:, csl], AF.Square)
                nc.vector.tensor_reduce(
                    na2_sb[:, cols],
                    sqb[:, :w * D].rearrange("p (t d) -> p t d", d=D),
                    axis=X, op=OP.add)
                act(u_sb[:, csl], u_sb[:, csl], AF.Relu)
                sqb2 = sqp.tile([P, Q * D], f32, tag="sqb")
                act(sqb2[:, :w * D], u_sb[:, csl], AF.Square)
                nc.vector.tensor_reduce(
                    nu2_sb[:, cols],
                    sqb2[:, :w * D].rearrange("p (t d) -> p t d", d=D),
                    axis=X, op=OP.add)
                # s' = min(1, R*/na);  rn = 1/nu;  t = nu*s';  A' = min(t, R*)
                sp = scp.tile([P, Q], f32, tag="sp")
                nc.vector.reciprocal(sp[:, :w], na2_sb[:, cols])
                act(sp[:, :w], sp[:, :w], AF.Sqrt, scale=R_STAR * R_STAR)
                nc.vector.tensor_scalar_min(sp[:, :w], sp[:, :w], 1.0)
                z2 = scp.tile([P, Q], f32, tag="z2")
                nc.vector.tensor_scalar_max(z2[:, :w], nu2_sb[:, cols], 1e-30)
                nc.vector.reciprocal(z2[:, :w], z2[:, :w])
                act(rn_sb[:, cols], z2[:, :w], AF.Sqrt)
                nu = scp.tile([P, Q], f32, tag="nu")
                nc.vector.tensor_tensor(nu[:, :w], nu2_sb[:, cols],
                                        rn_sb[:, cols], op=OP.mult)
                tq = scp.tile([P, Q], f32, tag="tq")
                nc.vector.tensor_tensor(tq[:, :w], nu[:, :w], sp[:, :w],
                                        op=OP.mult)
                if last_layer:
                    nc.vector.tensor_scalar_min(t2_sb[:, cols], tq[:, :w],
                                                MAX_TANH)
                else:
                    nc.vector.tensor_scalar_min(A_sb[:, cols], tq[:, :w],
                                                R_STAR)
                    nc.vector.tensor_tensor(y_sb[:, cols], A_sb[:, cols],
                                            rn_sb[:, cols], op=OP.mult)

            # chain-group boundaries: big groups early (hidden under DMA),
            # small groups at the tail to shrink the exposed serial end
            GROUPS = [(0, 8), (8, 16), (16, 24), (24, 28), (28, 30),
                      (30, 31), (31, 32)]
            def gather_layer(layer):
                """aggregate messages; per finished chain-group run the
                post-agg chain and the next stage's linear work."""
                row0_sb = scp.tile([1, D], f32, tag="row0")
                ioff = woff = 0
                gidx = 0
                for t in range(T):
                    K = int(Ks[t])
                    g = gp.tile([P, K * D], f32, tag="G")
                    g3 = g[:].rearrange("p (k d) -> p k d", d=D)
                    nc.gpsimd.dma_gather(
                        g3, xt_table[:, :], idx_sb[:, ioff:ioff + 8 * K],
                        num_idxs=P * K, num_idxs_reg=P * K, elem_size=D,
                        single_packet=False)
                    if t == 0 and not use_wt:
                        # row0 value for the pad-subtract; queued behind the
                        # first gather so it doesn't delay the pipeline start
                        nc.sync.dma_start(out=row0_sb[:], in_=xt_table[0:1, :])
                    if use_wt:
                        wt_ap = wt_sb[:, woff:woff + K]
                        wv = bass.AP(wt_ap.tensor, wt_ap.offset,
                                     list(wt_ap.ap) + [[0, D]])
                        nc.vector.tensor_tensor(g3, g3, wv, op=OP.mult)
                    nc.vector.tensor_reduce(
                        u_sb[:, ts(t)],
                        g[:].rearrange("p (k d) -> p d k", d=D),
                        axis=X, op=OP.add)
                    if not use_wt:
                        corr_ps = psC.tile([P, D], f32, tag="corr")
                        nc.tensor.matmul(
                            out=corr_ps[:], lhsT=pc_sb[0:1, t * P:(t + 1) * P],
                            rhs=row0_sb[0:1, :], start=True, stop=True)
                        nc.vector.tensor_tensor(u_sb[:, ts(t)], u_sb[:, ts(t)],
                                                corr_ps[:], op=OP.subtract)
                    ioff += 8 * K
                    woff += K
                    if t + 1 == GROUPS[gidx][1]:
                        g0, g1 = GROUPS[gidx]
                        gidx += 1
                        post_agg_group(g0, g1, last_layer=(layer == 1))
                        if layer == 0:
                            linear_block(w1t_sb, slice(g0, g1))
                            if g1 == 28:
                                # bulk of the next table can ship while the
                                # tail groups are still being chained
                                allgather(0, 28)
                        else:
                            readout_block(slice(g0, g1))

            # ---- layer 1 linear + publish -----------------------------------
            for h in range(2):
                intro_quarter(2 * h)
                intro_quarter(2 * h + 1)
                linear_block(w0t_sb, slice(16 * h, 16 * (h + 1)))
                allgather(16 * h, 16 * (h + 1))
            gather_layer(0)
            allgather(28, 32)
            gather_layer(1)

            # ---- tail: single Tanh, final scale broadcast, store ------------
            th = scp.tile([P, T], f32, tag="th")
            act(th[:], t2_sb[:], AF.Tanh)
            nc.vector.tensor_scalar_min(th[:], th[:], MAXN)
            nc.vector.tensor_tensor(sf_sb[:], th[:], rn_sb[:], op=OP.mult)
            o3 = out_sb[:].rearrange("p (t j) -> p t j", j=16)
            sf_ap = sf_sb[:]
            sfv = bass.AP(sf_ap.tensor, sf_ap.offset, list(sf_ap.ap) + [[0, 16]])
            nc.vector.tensor_tensor(o3, o3, sfv, op=OP.mult)
            nc.sync.dma_start(out=out_dram[:], in_=out_sb[:])
    nc.compile()
    return nc


def kernel(h, distances, rows, cols, node_mask, edge_mask,
           W0, b0, W1, b1, W_out, b_out, _trace=False):
    from concourse.bass_utils import run_bass_kernel_spmd

    h = np.asarray(h, dtype=np.float32)
    rows = np.asarray(rows).astype(np.int64)
    cols = np.asarray(cols).astype(np.int64)
    node_mask = np.asarray(node_mask, dtype=np.float32)
    edge_mask = np.asarray(edge_mask, dtype=np.float32)
    assert not np.any(np.asarray(b0)) and not np.any(np.asarray(b1)) and \
        not np.any(np.asarray(b_out)), "nonzero biases unsupported"

    perm, Ks, idx_dev, wt_dev, IDXC, WTC, pc_dev, allones = _build_tables(
        rows, cols, edge_mask, node_mask)

    hp = h[perm].reshape(C, T, P, D).transpose(0, 2, 1, 3).reshape(C, P, T * D)
    w0t = np.ascontiguousarray(np.vstack([np.asarray(W0, np.float32).T] * 2))
    w1t = np.ascontiguousarray(np.vstack([np.asarray(W1, np.float32).T] * 2))
    wot = np.ascontiguousarray(np.vstack([np.asarray(W_out, np.float32).T] * 2))

    nc = _build_program(Ks, IDXC, WTC, use_wt=not allones)
    in_maps = [{
        "h_in": np.ascontiguousarray(hp[c]),
        "idx_in": idx_dev[c],
        "wt_in": wt_dev[c],
        "w0t_in": w0t, "w1t_in": w1t, "wot_in": wot,
        "pc_in": pc_dev[c],
    } for c in range(C)]
    res = run_bass_kernel_spmd(nc, in_maps, list(range(C)), trace=_trace)
    od = np.stack([res.results[c]["out"] for c in range(C)])
    od = od.reshape(C, P, T, 16).transpose(0, 2, 1, 3).reshape(N, 16)
    out = np.empty((N, 16), np.float32)
    out[perm] = od
    if _trace:
        return out, res
    return out


# revision 52
# speedup vs baseline: 1.0237x; 1.0237x over previous
"""HGCN decoder on 8 trn2 NeuronCores.

Strategy: nodes are sorted by in-degree, grouped into 128-node tiles, and the
tiles are dealt round-robin across the 8 cores (graph-parallel by destination
node).  Each core:
  - runs the node-wise hyperbolic math on its 4096 nodes,
  - publishes its tangent-space table shard, AllGathers the full [32768, 64]
    table to DRAM,
  - aggregates messages with `dma_gather` (padded per-tile CSR) followed by a
    strided reduce on the vector engine,
  - finishes with the euclidean readout matmul.

Key algebraic simplifications vs the literal reference (all exact up to fp
rounding, validated against the jax reference):
  - proj-then-logmap0 collapses: artanh(min(tanh(r), 1-eps)) == min(r, R*)
    with R* = artanh(1-eps).  This removes every Ln/artanh evaluation and all
    but one final Tanh, so the scalar chains are pure min/mult/recip/sqrt.
  - positive per-node scales commute through relu and matmuls, so the vector
    state is kept UNSCALED (raw) and only combined per-node scalars are
    carried between stages.  The only materialized scalings are the published
    message table (xt = mv_raw * sigma) and the final readout.
Activation-table note: the whole kernel needs only {Square, Sqrt, Relu, Copy}
(one table) plus a single trailing Tanh (second table) => 2 table loads total.

All graph preprocessing happens host-side in numpy; the device only sees
dense tables.
"""

import numpy as np

N = 32768
E = 1015808
D = 64
C = 8          # cores
NL = N // C    # 4096 nodes per core
P = 128        # partitions / tile
T = NL // P    # 32 tiles per core
Q = 8          # tiles per chain-quarter
NQ = T // Q
MAXN = 1.0 - 4e-3   # PROJ_EPS boundary for c=1
EPS = 1e-15
MAX_TANH = 15.0
R_STAR = float(np.arctanh(np.float64(1.0 - 4e-3)))  # artanh(MAXN) = 3.10635...


def _build_tables(rows, cols, edge_mask, node_mask):
    """Permute nodes by degree, deal tiles round-robin to cores, and build the
    per-core padded gather tables (int16 indices wrapped the way
    InstDMAGatherAnt wants them) plus matching weight tables."""
    deg = np.bincount(rows, minlength=N)
    order = np.argsort(-deg, kind="stable")
    # global tile j -> core j%C, sorted-slot j//C; slots are then re-ordered so
    # the smallest-K slot runs FIRST (cheap desc-gen before the pipeline
    # fills) and the second-smallest LAST (short exposed tail):
    #   slot order = [smallest, biggest, ..., 2nd smallest]
    S = np.empty(T, dtype=np.int64)
    S[0] = T - 1
    S[1:] = np.arange(T - 1)
    Sinv = np.empty(T, dtype=np.int64)
    Sinv[S] = np.arange(T)
    perm = np.empty(N, dtype=np.int64)
    j = np.arange(N) // P                     # global tile of sorted rank r
    c = j % C
    t = Sinv[j // C]
    p = np.arange(N) % P
    perm[c * NL + t * P + p] = order          # perm[g] = original node id
    pos = np.empty(N, dtype=np.int64)
    pos[perm] = np.arange(N)

    # gather-table row id for permuted position g=(c,t,p):
    #   AllGather concatenates per-core [P, T*D] blocks, so
    #   row_id = c*NL + p*T + t
    gg = np.arange(N)
    gc, gr = gg // NL, gg % NL
    gt, gp_ = gr // P, gr % P
    rowid = gc * NL + gp_ * T + gt            # [g] -> table row
    dstpos = pos[rows]
    eorder = np.argsort(dstpos, kind="stable")
    src_sorted = rowid[pos[cols[eorder]]]     # gather table rows, 0..N-1
    w_sorted = edge_mask[eorder, 0].astype(np.float64)
    cnts = np.bincount(dstpos, minlength=N)
    offs = np.zeros(N + 1, dtype=np.int64)
    np.cumsum(cnts, out=offs[1:])

    # per-slot K: max count over the 8 cores' tiles in that slot
    cnts_g = cnts.reshape(C, T, P)
    Ks = np.maximum(cnts_g.max(axis=(0, 2)), 1).astype(np.int64)   # [T]

    IDXC = int(8 * Ks.sum())
    WTC = int(Ks.sum())
    idx_dev = np.zeros((C, P, IDXC), np.int16)
    wt_dev = np.zeros((C, P, WTC), np.float32)
    nm = node_mask[:, 0].astype(np.float64)
    ioff = woff = 0
    ar = None
    for t in range(T):
        K = int(Ks[t])
        if ar is None or ar.shape[1] != K:
            ar = np.arange(K)[None, :]
        for cc in range(C):
            base = cc * NL + t * P
            cn = cnts[base:base + P]
            take = offs[base:base + P][:, None] + ar          # [P, K]
            valid = ar < cn[:, None]
            take_c = np.minimum(take, E - 1)
            nb = np.where(valid, src_sorted[take_c], 0)
            wl = np.where(valid, w_sorted[take_c], 0.0)
            wl = wl * nm[perm[base:base + P]][:, None]
            il = nb.T.reshape(-1)                             # i = g*128+p
            ch = il.reshape(8 * K, 16).T                      # [16, 8K]
            idx_dev[cc, :, ioff:ioff + 8 * K] = np.tile(ch, (8, 1)).astype(np.int16)
            wt_dev[cc, :, woff:woff + K] = wl.astype(np.float32)
        ioff += 8 * K
        woff += K
    # pad counts per (core, slot, partition) for the pad-subtract path
    pc_dev = np.zeros((C, 1, T * P), np.float32)
    for t in range(T):
        K = int(Ks[t])
        for cc in range(C):
            base = cc * NL + t * P
            pc_dev[cc, 0, t * P:(t + 1) * P] = K - cnts[base:base + P]
    allones = bool(np.all(edge_mask == 1.0) and np.all(node_mask == 1.0))
    return perm, Ks, idx_dev, wt_dev, IDXC, WTC, pc_dev, allones


def _build_program(Ks, IDXC, WTC, use_wt=True, sim=False):
    import concourse.bacc as bacc
    import concourse.bass as bass
    import concourse.mybir as mybir
    import concourse.tile as tile
    from concourse import library_config
    from concourse.masks import make_identity

    f32 = mybir.dt.float32
    i16 = mybir.dt.int16
    AF = mybir.ActivationFunctionType
    OP = mybir.AluOpType
    X = mybir.AxisListType.X

    nc = bacc.Bacc("TRN2", target_bir_lowering=False, debug=False,
                   num_devices=1 if sim else C)

    h_in = nc.dram_tensor("h_in", [P, T * D], f32, kind="ExternalInput")
    idx_in = nc.dram_tensor("idx_in", [P, IDXC], i16, kind="ExternalInput")
    wt_in = nc.dram_tensor("wt_in", [P, WTC], f32, kind="ExternalInput")
    w0t_in = nc.dram_tensor("w0t_in", [2 * D, D], f32, kind="ExternalInput")
    w1t_in = nc.dram_tensor("w1t_in", [2 * D, D], f32, kind="ExternalInput")
    wot_in = nc.dram_tensor("wot_in", [2 * D, 16], f32, kind="ExternalInput")
    pc_in = nc.dram_tensor("pc_in", [1, T * P], f32, kind="ExternalInput")
    out_dram = nc.dram_tensor("out", [P, T * 16], f32, kind="ExternalOutput")
    xt_shard = nc.dram_tensor("xt_shard", [P, T * D], f32)
    xt_table = nc.dram_tensor("xt_table", [N, D], f32, addr_space="Shared")
    groups = [list(range(C))]

    with tile.TileContext(nc) as tc:
        nc.gpsimd.load_library(library_config.mlp)
        import contextlib
        ctx = contextlib.ExitStack()
        with ctx:
            const = ctx.enter_context(tc.tile_pool(name="const", bufs=1))
            sqp = ctx.enter_context(tc.tile_pool(name="sq", bufs=2))
            xtp = ctx.enter_context(tc.tile_pool(name="xtp", bufs=3))
            gp = ctx.enter_context(tc.tile_pool(name="gp", bufs=4))
            scp = ctx.enter_context(tc.tile_pool(name="scp", bufs=3))
            psT = ctx.enter_context(tc.tile_pool(name="psT", bufs=2, space="PSUM"))
            psM = ctx.enter_context(tc.tile_pool(name="psM", bufs=2, space="PSUM"))
            psC = ctx.enter_context(tc.tile_pool(name="psC", bufs=2, space="PSUM"))

            ident = const.tile([P, P], f32)
            make_identity(nc, ident[:])
            # weights duplicated across both partition halves so matmuls can
            # source lhsT from either half of a paired transpose
            w0t_sb = const.tile([2 * D, D], f32)
            w1t_sb = const.tile([2 * D, D], f32)
            wot_sb = const.tile([2 * D, 16], f32)
            idx_sb = const.tile([P, IDXC], i16)
            pc_sb = const.tile([1, T * P], f32)
            if use_wt:
                wt_sb = const.tile([P, WTC], f32)

            u_sb = const.tile([P, T * D], f32)      # raw vector state / agg
            mv_sb = const.tile([P, T * D], f32)     # W@u then scaled messages
            out_sb = const.tile([P, T * 16], f32)
            # long-lived per-node scalars, [P, T] column t = tile t
            A_sb = const.tile([P, T], f32)    # artanh(||x||) of current state
            rn_sb = const.tile([P, T], f32)   # 1 / ||u_raw||
            y_sb = const.tile([P, T], f32)    # A * rn (sigma chain operand)
            t2_sb = const.tile([P, T], f32)   # final tanh argument (layer 2)
            na2_sb = const.tile([P, T], f32)
            nu2_sb = const.tile([P, T], f32)
            mx2_sb = const.tile([P, T], f32)
            sg_sb = const.tile([P, T], f32)   # publish scale sigma per tile
            sf_sb = const.tile([P, T], f32)   # final readout scale

            # load order: h quarter 0 + W0 first (intro/linear need them), the
            # gather metadata afterwards so it streams under the intro compute
            QW = Q * D
            nc.sync.dma_start(out=u_sb[:, 0:QW], in_=h_in[:, 0:QW])
            nc.sync.dma_start(out=w0t_sb[:], in_=w0t_in[:])
            for q in range(1, NQ):
                nc.sync.dma_start(out=u_sb[:, q * QW:(q + 1) * QW],
                                  in_=h_in[:, q * QW:(q + 1) * QW])
            nc.sync.dma_start(out=w1t_sb[:], in_=w1t_in[:])
            nc.sync.dma_start(out=wot_sb[:], in_=wot_in[:])
            nc.sync.dma_start(out=pc_sb[:], in_=pc_in[:])
            nc.sync.dma_start(out=idx_sb[:], in_=idx_in[:])
            if use_wt:
                nc.sync.dma_start(out=wt_sb[:], in_=wt_in[:])

            def ts(t, w=D):
                return slice(t * w, (t + 1) * w)

            def act(out, in_, f, **kw):
                nc.scalar.activation(out, in_, f, **kw)

            def intro_quarter(q):
                """A = min(||h||, R*), rn = 1/||h||, y = A*rn for quarter q."""
                cols = ts(q, Q)
                n02 = scp.tile([P, Q], f32, tag="n02")
                sqb = sqp.tile([P, Q * D], f32, tag="sqb")
                act(sqb[:], u_sb[:, ts(q, Q * D)], AF.Square)
                nc.vector.tensor_reduce(
                    n02[:], sqb[:].rearrange("p (t d) -> p t d", d=D),
                    axis=X, op=OP.add)
                z = scp.tile([P, Q], f32, tag="z0")
                nc.vector.reciprocal(z[:], n02[:])
                act(rn_sb[:, cols], z[:], AF.Sqrt)          # 1/n0
                n0 = scp.tile([P, Q], f32, tag="n0")
                nc.vector.tensor_tensor(n0[:], n02[:], rn_sb[:, cols],
                                        op=OP.mult)         # n0
                nc.vector.tensor_scalar_min(A_sb[:, cols], n0[:], R_STAR)
                nc.vector.tensor_tensor(y_sb[:, cols], A_sb[:, cols],
                                        rn_sb[:, cols], op=OP.mult)

            def mm_tiles(tiles, w_sb, dst_sb, ow):
                """transpose+matmul u[tiles] @ W^T -> dst_sb (free width ow),
                batching up to 4 matmul outputs per PSUM tile/copy."""
                t0, t1 = tiles.start, tiles.stop
                for tt in range(t0, t1):
                    j = tt - t0
                    if j % 2 == 0:
                        np2 = min(2, t1 - tt)
                        ps = psT.tile([P, P], f32, tag="xT2")
                        nc.tensor.transpose(
                            out=ps[0:np2 * D, :],
                            in_=u_sb[:, tt * D:(tt + np2) * D],
                            identity=ident[:])
                        xT2 = xtp.tile([P, P], f32, tag="xT2sb")
                        nc.vector.tensor_copy(xT2[0:np2 * D, :],
                                              ps[0:np2 * D, :])
                    h2 = j % 2
                    pm = psM.tile([P, ow], f32, tag="mv")
                    nc.tensor.matmul(out=pm[:],
                                     lhsT=xT2[h2 * D:(h2 + 1) * D, :],
                                     rhs=w_sb[h2 * D:(h2 + 1) * D, :],
                                     start=True, stop=True)
                    act(dst_sb[:, tt * ow:(tt + 1) * ow], pm[:], AF.Copy)

            def linear_block(w_sb, tiles, publish=True):
                """mv_raw = u @ W^T for `tiles`; sigma = min(y, R*/mraw);
                mv *= sigma; publish.  (y = A*rn precomputed per node.)"""
                t0, t1 = tiles.start, tiles.stop
                nt = t1 - t0
                mm_tiles(tiles, w_sb, mv_sb, D)
                # batched ||mv||^2 then the fused sigma chain
                for j in range((nt + Q - 1) // Q):
                    c0 = t0 + j * Q
                    c1 = min(c0 + Q, t1)
                    sqb = sqp.tile([P, Q * D], f32, tag="sqb")
                    act(sqb[:, :(c1 - c0) * D], mv_sb[:, c0 * D:c1 * D], AF.Square)
                    nc.vector.tensor_reduce(
                        mx2_sb[:, c0:c1],
                        sqb[:, :(c1 - c0) * D].rearrange("p (t d) -> p t d", d=D),
                        axis=X, op=OP.add)
                cols = slice(t0, t1)
                rm = scp.tile([P, T], f32, tag="rm")
                nc.vector.reciprocal(rm[:, cols], mx2_sb[:, cols])
                act(rm[:, cols], rm[:, cols], AF.Sqrt, scale=R_STAR * R_STAR)
                nc.vector.tensor_tensor(sg_sb[:, cols], y_sb[:, cols],
                                        rm[:, cols], op=OP.min)
                # mv *= sigma, broadcast over d in one strided multiply
                mv3 = mv_sb[:, t0 * D:t1 * D].rearrange("p (t d) -> p t d", d=D)
                sg_ap = sg_sb[:, cols]
                sgv = bass.AP(sg_ap.tensor, sg_ap.offset,
                              list(sg_ap.ap) + [[0, D]])
                nc.vector.tensor_tensor(mv3, mv3, sgv, op=OP.mult)
                if publish:
                    nc.sync.dma_start(out=xt_shard[:, t0 * D:t1 * D],
                                      in_=mv_sb[:, t0 * D:t1 * D])

            def readout_block(tiles):
                mm_tiles(tiles, wot_sb, out_sb, 16)

            def allgather():
                tc.strict_bb_all_engine_barrier()
                if sim:
                    nc.sync.dma_start(
                        out=xt_table[0:NL, :].rearrange("(p x) d -> p x d", p=P),
                        in_=xt_shard[:].rearrange("p (x d) -> p x d", d=D))
                else:
                    nc.gpsimd.collective_compute(
                        "AllGather", mybir.AluOpType.bypass, replica_groups=groups,
                        ins=[xt_shard[:, :]], outs=[xt_table[:, :]])
                tc.strict_bb_all_engine_barrier()

            def post_agg_group(g0, g1, last_layer):
                """norms of agg & relu(agg); s' chain; t = nu*s'; A'/rn' (or
                stash t2 for the final tanh)."""
                w = g1 - g0
                cols = slice(g0, g1)
                csl = slice(g0 * D, g1 * D)
                sqb = sqp.tile([P, Q * D], f32, tag="sqb")
                act(sqb[:, :w * D], u_sb[:, csl], AF.Square)
                nc.vector.tensor_reduce(
                    na2_sb[:, cols],
                    sqb[:, :w * D].rearrange("p (t d) -> p t d", d=D),
                    axis=X, op=OP.add)
                act(u_sb[:, csl], u_sb[:, csl], AF.Relu)
                sqb2 = sqp.tile([P, Q * D], f32, tag="sqb")
                act(sqb2[:, :w * D], u_sb[:, csl], AF.Square)
                nc.vector.tensor_reduce(
                    nu2_sb[:, cols],
                    sqb2[:, :w * D].rearrange("p (t d) -> p t d", d=D),
                    axis=X, op=OP.add)
                # s' = min(1, R*/na);  rn = 1/nu;  t = nu*s';  A' = min(t, R*)
                sp = scp.tile([P, Q], f32, tag="sp")
                nc.vector.reciprocal(sp[:, :w], na2_sb[:, cols])
                act(sp[:, :w], sp[:, :w], AF.Sqrt, scale=R_STAR * R_STAR)
                nc.vector.tensor_scalar_min(sp[:, :w], sp[:, :w], 1.0)
                z2 = scp.tile([P, Q], f32, tag="z2")
                nc.vector.tensor_scalar_max(z2[:, :w], nu2_sb[:, cols], 1e-30)
                nc.vector.reciprocal(z2[:, :w], z2[:, :w])
                act(rn_sb[:, cols], z2[:, :w], AF.Sqrt)
                nu = scp.tile([P, Q], f32, tag="nu")
                nc.vector.tensor_tensor(nu[:, :w], nu2_sb[:, cols],
                                        rn_sb[:, cols], op=OP.mult)
                tq = scp.tile([P, Q], f32, tag="tq")
                nc.vector.tensor_tensor(tq[:, :w], nu[:, :w], sp[:, :w],
                                        op=OP.mult)
                if last_layer:
                    nc.vector.tensor_scalar_min(t2_sb[:, cols], tq[:, :w],
                                                MAX_TANH)
                else:
                    nc.vector.tensor_scalar_min(A_sb[:, cols], tq[:, :w],
                                                R_STAR)
                    nc.vector.tensor_tensor(y_sb[:, cols], A_sb[:, cols],
                                            rn_sb[:, cols], op=OP.mult)

            # chain-group boundaries: big groups early (hidden under DMA),
            # small groups at the tail to shrink the exposed serial end
            GROUPS = [(0, 8), (8, 16), (16, 24), (24, 28), (28, 30),
                      (30, 31), (31, 32)]
            def gather_layer(layer):
                """aggregate messages; per finished chain-group run the
                post-agg chain and the next stage's linear work."""
                row0_sb = scp.tile([1, D], f32, tag="row0")
                ioff = woff = 0
                gidx = 0
                for t in range(T):
                    K = int(Ks[t])
                    g = gp.tile([P, K * D], f32, tag="G")
                    g3 = g[:].rearrange("p (k d) -> p k d", d=D)
                    nc.gpsimd.dma_gather(
                        g3, xt_table[:, :], idx_sb[:, ioff:ioff + 8 * K],
                        num_idxs=P * K, num_idxs_reg=P * K, elem_size=D,
                        single_packet=False)
                    if t == 0 and not use_wt:
                        # row0 value for the pad-subtract; queued behind the
                        # first gather so it doesn't delay the pipeline start
                        nc.sync.dma_start(out=row0_sb[:], in_=xt_table[0:1, :])
                    if use_wt:
                        wt_ap = wt_sb[:, woff:woff + K]
                        wv = bass.AP(wt_ap.tensor, wt_ap.offset,
                                     list(wt_ap.ap) + [[0, D]])
                        nc.vector.tensor_tensor(g3, g3, wv, op=OP.mult)
                    nc.vector.tensor_reduce(
                        u_sb[:, ts(t)],
                        g[:].rearrange("p (k d) -> p d k", d=D),
                        axis=X, op=OP.add)
                    if not use_wt:
                        corr_ps = psC.tile([P, D], f32, tag="corr")
                        nc.tensor.matmul(
                            out=corr_ps[:], lhsT=pc_sb[0:1, t * P:(t + 1) * P],
                            rhs=row0_sb[0:1, :], start=True, stop=True)
                        nc.vector.tensor_tensor(u_sb[:, ts(t)], u_sb[:, ts(t)],
                                                corr_ps[:], op=OP.subtract)
                    ioff += 8 * K
                    woff += K
                    if t + 1 == GROUPS[gidx][1]:
                        g0, g1 = GROUPS[gidx]
                        gidx += 1
                        post_agg_group(g0, g1, last_layer=(layer == 1))
                        if layer == 0:
                            linear_block(w1t_sb, slice(g0, g1))
                        else:
                            readout_block(slice(g0, g1))

            # ---- layer 1 linear + publish -----------------------------------
            for h in range(2):
                intro_quarter(2 * h)
                intro_quarter(2 * h + 1)
                linear_block(w0t_sb, slice(16 * h, 16 * (h + 1)))
            allgather()
            gather_layer(0)
            allgather()
            gather_layer(1)

            # ---- tail: single Tanh, final scale broadcast, store ------------
            th = scp.tile([P, T], f32, tag="th")
            act(th[:], t2_sb[:], AF.Tanh)
            nc.vector.tensor_scalar_min(th[:], th[:], MAXN)
            nc.vector.tensor_tensor(sf_sb[:], th[:], rn_sb[:], op=OP.mult)
            o3 = out_sb[:].rearrange("p (t j) -> p t j", j=16)
            sf_ap = sf_sb[:]
            sfv = bass.AP(sf_ap.tensor, sf_ap.offset, list(sf_ap.ap) + [[0, 16]])
            nc.vector.tensor_tensor(o3, o3, sfv, op=OP.mult)
            nc.sync.dma_start(out=out_dram[:], in_=out_sb[:])
    nc.compile()
    return nc


def kernel(h, distances, rows, cols, node_mask, edge_mask,
           W0, b0, W1, b1, W_out, b_out, _trace=False):
    from concourse.bass_utils import run_bass_kernel_spmd

    h = np.asarray(h, dtype=np.float32)
    rows = np.asarray(rows).astype(np.int64)
    cols = np.asarray(cols).astype(np.int64)
    node_mask = np.asarray(node_mask, dtype=np.float32)
    edge_mask = np.asarray(edge_mask, dtype=np.float32)
    assert not np.any(np.asarray(b0)) and not np.any(np.asarray(b1)) and \
        not np.any(np.asarray(b_out)), "nonzero biases unsupported"

    perm, Ks, idx_dev, wt_dev, IDXC, WTC, pc_dev, allones = _build_tables(
        rows, cols, edge_mask, node_mask)

    hp = h[perm].reshape(C, T, P, D).transpose(0, 2, 1, 3).reshape(C, P, T * D)
    w0t = np.ascontiguousarray(np.vstack([np.asarray(W0, np.float32).T] * 2))
    w1t = np.ascontiguousarray(np.vstack([np.asarray(W1, np.float32).T] * 2))
    wot = np.ascontiguousarray(np.vstack([np.asarray(W_out, np.float32).T] * 2))

    nc = _build_program(Ks, IDXC, WTC, use_wt=not allones)
    in_maps = [{
        "h_in": np.ascontiguousarray(hp[c]),
        "idx_in": idx_dev[c],
        "wt_in": wt_dev[c],
        "w0t_in": w0t, "w1t_in": w1t, "wot_in": wot,
        "pc_in": pc_dev[c],
    } for c in range(C)]
    res = run_bass_kernel_spmd(nc, in_maps, list(range(C)), trace=_trace)
    od = np.stack([res.results[c]["out"] for c in range(C)])
    od = od.reshape(C, P, T, 16).transpose(0, 2, 1, 3).reshape(N, 16)
    out = np.empty((N, 16), np.float32)
    out[perm] = od
    if _trace:
        return out, res
    return out


# revision 55
# speedup vs baseline: 1.0261x; 1.0023x over previous
"""HGCN decoder on 8 trn2 NeuronCores.

Strategy: nodes are sorted by in-degree, grouped into 128-node tiles, and the
tiles are dealt round-robin across the 8 cores (graph-parallel by destination
node).  Each core:
  - runs the node-wise hyperbolic math on its 4096 nodes,
  - publishes its tangent-space table shard, AllGathers the full [32768, 64]
    table to DRAM,
  - aggregates messages with `dma_gather` (padded per-tile CSR) followed by a
    strided reduce on the vector engine,
  - finishes with the euclidean readout matmul.

Key algebraic simplifications vs the literal reference (all exact up to fp
rounding, validated against the jax reference):
  - proj-then-logmap0 collapses: artanh(min(tanh(r), 1-eps)) == min(r, R*)
    with R* = artanh(1-eps).  This removes every Ln/artanh evaluation and all
    but one final Tanh, so the scalar chains are pure min/mult/recip/sqrt.
  - positive per-node scales commute through relu and matmuls, so the vector
    state is kept UNSCALED (raw) and only combined per-node scalars are
    carried between stages.  The only materialized scalings are the published
    message table (xt = mv_raw * sigma) and the final readout.
Activation-table note: the whole kernel needs only {Square, Sqrt, Relu, Copy}
(one table) plus a single trailing Tanh (second table) => 2 table loads total.

All graph preprocessing happens host-side in numpy; the device only sees
dense tables.
"""

import numpy as np

N = 32768
E = 1015808
D = 64
C = 8          # cores
NL = N // C    # 4096 nodes per core
P = 128        # partitions / tile
T = NL // P    # 32 tiles per core
Q = 8          # tiles per chain-quarter
NQ = T // Q
MAXN = 1.0 - 4e-3   # PROJ_EPS boundary for c=1
EPS = 1e-15
MAX_TANH = 15.0
R_STAR = float(np.arctanh(np.float64(1.0 - 4e-3)))  # artanh(MAXN) = 3.10635...


def _build_tables(rows, cols, edge_mask, node_mask):
    """Permute nodes by degree, deal tiles round-robin to cores, and build the
    per-core padded gather tables (int16 indices wrapped the way
    InstDMAGatherAnt wants them) plus matching weight tables."""
    deg = np.bincount(rows, minlength=N)
    order = np.argsort(-deg, kind="stable")
    # global tile j -> core j%C, sorted-slot j//C; slots are then re-ordered so
    # the smallest-K slot runs FIRST (cheap desc-gen before the pipeline
    # fills) and the second-smallest LAST (short exposed tail):
    #   slot order = [smallest, biggest, ..., 2nd smallest]
    S = np.empty(T, dtype=np.int64)
    S[0] = T - 1
    S[1:] = np.arange(T - 1)
    Sinv = np.empty(T, dtype=np.int64)
    Sinv[S] = np.arange(T)
    perm = np.empty(N, dtype=np.int64)
    j = np.arange(N) // P                     # global tile of sorted rank r
    c = j % C
    t = Sinv[j // C]
    p = np.arange(N) % P
    perm[c * NL + t * P + p] = order          # perm[g] = original node id
    pos = np.empty(N, dtype=np.int64)
    pos[perm] = np.arange(N)

    # gather-table row id for permuted position g=(c,t,p):
    #   AllGather concatenates per-core [P, T*D] blocks, so
    #   row_id = c*NL + p*T + t
    gg = np.arange(N)
    gc, gr = gg // NL, gg % NL
    gt, gp_ = gr // P, gr % P
    rowid = gc * NL + gp_ * T + gt            # [g] -> table row
    dstpos = pos[rows]
    eorder = np.argsort(dstpos, kind="stable")
    src_sorted = rowid[pos[cols[eorder]]]     # gather table rows, 0..N-1
    w_sorted = edge_mask[eorder, 0].astype(np.float64)
    cnts = np.bincount(dstpos, minlength=N)
    offs = np.zeros(N + 1, dtype=np.int64)
    np.cumsum(cnts, out=offs[1:])

    # per-slot K: max count over the 8 cores' tiles in that slot
    cnts_g = cnts.reshape(C, T, P)
    Ks = np.maximum(cnts_g.max(axis=(0, 2)), 1).astype(np.int64)   # [T]

    IDXC = int(8 * Ks.sum())
    WTC = int(Ks.sum())
    idx_dev = np.zeros((C, P, IDXC), np.int16)
    wt_dev = np.zeros((C, P, WTC), np.float32)
    nm = node_mask[:, 0].astype(np.float64)
    ioff = woff = 0
    ar = None
    for t in range(T):
        K = int(Ks[t])
        if ar is None or ar.shape[1] != K:
            ar = np.arange(K)[None, :]
        for cc in range(C):
            base = cc * NL + t * P
            cn = cnts[base:base + P]
            take = offs[base:base + P][:, None] + ar          # [P, K]
            valid = ar < cn[:, None]
            take_c = np.minimum(take, E - 1)
            nb = np.where(valid, src_sorted[take_c], 0)
            wl = np.where(valid, w_sorted[take_c], 0.0)
            wl = wl * nm[perm[base:base + P]][:, None]
            il = nb.T.reshape(-1)                             # i = g*128+p
            ch = il.reshape(8 * K, 16).T                      # [16, 8K]
            idx_dev[cc, :, ioff:ioff + 8 * K] = np.tile(ch, (8, 1)).astype(np.int16)
            wt_dev[cc, :, woff:woff + K] = wl.astype(np.float32)
        ioff += 8 * K
        woff += K
    # pad counts per (core, slot, partition) for the pad-subtract path
    pc_dev = np.zeros((C, 1, T * P), np.float32)
    for t in range(T):
        K = int(Ks[t])
        for cc in range(C):
            base = cc * NL + t * P
            pc_dev[cc, 0, t * P:(t + 1) * P] = K - cnts[base:base + P]
    allones = bool(np.all(edge_mask == 1.0) and np.all(node_mask == 1.0))
    return perm, Ks, idx_dev, wt_dev, IDXC, WTC, pc_dev, allones


def _build_program(Ks, IDXC, WTC, use_wt=True, sim=False):
    import concourse.bacc as bacc
    import concourse.bass as bass
    import concourse.mybir as mybir
    import concourse.tile as tile
    from concourse import library_config
    from concourse.masks import make_identity

    f32 = mybir.dt.float32
    i16 = mybir.dt.int16
    AF = mybir.ActivationFunctionType
    OP = mybir.AluOpType
    X = mybir.AxisListType.X

    nc = bacc.Bacc("TRN2", target_bir_lowering=False, debug=False,
                   num_devices=1 if sim else C)

    h_in = nc.dram_tensor("h_in", [P, T * D], f32, kind="ExternalInput")
    idx_in = nc.dram_tensor("idx_in", [P, IDXC], i16, kind="ExternalInput")
    wt_in = nc.dram_tensor("wt_in", [P, WTC], f32, kind="ExternalInput")
    w0t_in = nc.dram_tensor("w0t_in", [2 * D, D], f32, kind="ExternalInput")
    w1t_in = nc.dram_tensor("w1t_in", [2 * D, D], f32, kind="ExternalInput")
    wot_in = nc.dram_tensor("wot_in", [2 * D, 16], f32, kind="ExternalInput")
    pc_in = nc.dram_tensor("pc_in", [1, T * P], f32, kind="ExternalInput")
    out_dram = nc.dram_tensor("out", [P, T * 16], f32, kind="ExternalOutput")
    xt_shard = nc.dram_tensor("xt_shard", [P, T * D], f32)
    xt_table = nc.dram_tensor("xt_table", [N, D], f32, addr_space="Shared")
    groups = [list(range(C))]

    with tile.TileContext(nc) as tc:
        nc.gpsimd.load_library(library_config.mlp)
        import contextlib
        ctx = contextlib.ExitStack()
        with ctx:
            const = ctx.enter_context(tc.tile_pool(name="const", bufs=1))
            sqp = ctx.enter_context(tc.tile_pool(name="sq", bufs=2))
            xtp = ctx.enter_context(tc.tile_pool(name="xtp", bufs=3))
            gp = ctx.enter_context(tc.tile_pool(name="gp", bufs=4))
            scp = ctx.enter_context(tc.tile_pool(name="scp", bufs=3))
            psT = ctx.enter_context(tc.tile_pool(name="psT", bufs=2, space="PSUM"))
            psM = ctx.enter_context(tc.tile_pool(name="psM", bufs=2, space="PSUM"))
            psC = ctx.enter_context(tc.tile_pool(name="psC", bufs=2, space="PSUM"))

            ident = const.tile([P, P], f32)
            make_identity(nc, ident[:])
            # weights duplicated across both partition halves so matmuls can
            # source lhsT from either half of a paired transpose
            w0t_sb = const.tile([2 * D, D], f32)
            w1t_sb = const.tile([2 * D, D], f32)
            wot_sb = const.tile([2 * D, 16], f32)
            idx_sb = const.tile([P, IDXC], i16)
            pc_sb = const.tile([1, T * P], f32)
            if use_wt:
                wt_sb = const.tile([P, WTC], f32)

            u_sb = const.tile([P, T * D], f32)      # raw vector state / agg
            mv_sb = const.tile([P, T * D], f32)     # W@u then scaled messages
            out_sb = const.tile([P, T * 16], f32)
            # long-lived per-node scalars, [P, T] column t = tile t
            A_sb = const.tile([P, T], f32)    # artanh(||x||) of current state
            rn_sb = const.tile([P, T], f32)   # 1 / ||u_raw||
            y_sb = const.tile([P, T], f32)    # A * rn (sigma chain operand)
            t2_sb = const.tile([P, T], f32)   # final tanh argument (layer 2)
            na2_sb = const.tile([P, T], f32)
            nu2_sb = const.tile([P, T], f32)
            mx2_sb = const.tile([P, T], f32)
            sg_sb = const.tile([P, T], f32)   # publish scale sigma per tile
            sf_sb = const.tile([P, T], f32)   # final readout scale

            # load order: h quarter 0 + W0 first (intro/linear need them), the
            # gather metadata afterwards so it streams under the intro compute
            QW = Q * D
            nc.sync.dma_start(out=u_sb[:, 0:QW], in_=h_in[:, 0:QW])
            nc.sync.dma_start(out=w0t_sb[:], in_=w0t_in[:])
            for q in range(1, NQ):
                nc.sync.dma_start(out=u_sb[:, q * QW:(q + 1) * QW],
                                  in_=h_in[:, q * QW:(q + 1) * QW])
            nc.sync.dma_start(out=w1t_sb[:], in_=w1t_in[:])
            nc.sync.dma_start(out=wot_sb[:], in_=wot_in[:])
            nc.sync.dma_start(out=pc_sb[:], in_=pc_in[:])
            nc.sync.dma_start(out=idx_sb[:], in_=idx_in[:])
            if use_wt:
                nc.sync.dma_start(out=wt_sb[:], in_=wt_in[:])

            def ts(t, w=D):
                return slice(t * w, (t + 1) * w)

            def act(out, in_, f, **kw):
                nc.scalar.activation(out, in_, f, **kw)

            def intro_quarter(q):
                """A = min(||h||, R*), rn = 1/||h||, y = A*rn for quarter q."""
                cols = ts(q, Q)
                n02 = scp.tile([P, Q], f32, tag="n02")
                sqb = sqp.tile([P, Q * D], f32, tag="sqb")
                act(sqb[:], u_sb[:, ts(q, Q * D)], AF.Square)
                nc.vector.tensor_reduce(
                    n02[:], sqb[:].rearrange("p (t d) -> p t d", d=D),
                    axis=X, op=OP.add)
                z = scp.tile([P, Q], f32, tag="z0")
                nc.vector.reciprocal(z[:], n02[:])
                act(rn_sb[:, cols], z[:], AF.Sqrt)          # 1/n0
                n0 = scp.tile([P, Q], f32, tag="n0")
                nc.vector.tensor_tensor(n0[:], n02[:], rn_sb[:, cols],
                                        op=OP.mult)         # n0
                nc.vector.tensor_scalar_min(A_sb[:, cols], n0[:], R_STAR)
                nc.vector.tensor_tensor(y_sb[:, cols], A_sb[:, cols],
                                        rn_sb[:, cols], op=OP.mult)

            def mm_tiles(tiles, w_sb, dst_sb, ow):
                """transpose+matmul u[tiles] @ W^T -> dst_sb (free width ow),
                batching up to 4 matmul outputs per PSUM tile/copy."""
                t0, t1 = tiles.start, tiles.stop
                for tt in range(t0, t1):
                    j = tt - t0
                    if j % 2 == 0:
                        np2 = min(2, t1 - tt)
                        ps = psT.tile([P, P], f32, tag="xT2")
                        nc.tensor.transpose(
                            out=ps[0:np2 * D, :],
                            in_=u_sb[:, tt * D:(tt + np2) * D],
                            identity=ident[:])
                        xT2 = xtp.tile([P, P], f32, tag="xT2sb")
                        nc.vector.tensor_copy(xT2[0:np2 * D, :],
                                              ps[0:np2 * D, :])
                    h2 = j % 2
                    # NOTE: matmul PSUM destinations must be whole pool tiles;
                    # column-sliced outputs crash the device (works in sim).
                    pm = psM.tile([P, ow], f32, tag="mv")
                    nc.tensor.matmul(out=pm[:],
                                     lhsT=xT2[h2 * D:(h2 + 1) * D, :],
                                     rhs=w_sb[h2 * D:(h2 + 1) * D, :],
                                     start=True, stop=True)
                    act(dst_sb[:, tt * ow:(tt + 1) * ow], pm[:], AF.Copy)

            def linear_block(w_sb, tiles, publish=True):
                """mv_raw = u @ W^T for `tiles`; sigma = min(y, R*/mraw);
                mv *= sigma; publish.  (y = A*rn precomputed per node.)"""
                t0, t1 = tiles.start, tiles.stop
                nt = t1 - t0
                mm_tiles(tiles, w_sb, mv_sb, D)
                # batched ||mv||^2 then the fused sigma chain
                for j in range((nt + Q - 1) // Q):
                    c0 = t0 + j * Q
                    c1 = min(c0 + Q, t1)
                    sqb = sqp.tile([P, Q * D], f32, tag="sqb")
                    act(sqb[:, :(c1 - c0) * D], mv_sb[:, c0 * D:c1 * D], AF.Square)
                    nc.vector.tensor_reduce(
                        mx2_sb[:, c0:c1],
                        sqb[:, :(c1 - c0) * D].rearrange("p (t d) -> p t d", d=D),
                        axis=X, op=OP.add)
                cols = slice(t0, t1)
                rm = scp.tile([P, T], f32, tag="rm")
                nc.vector.reciprocal(rm[:, cols], mx2_sb[:, cols])
                act(rm[:, cols], rm[:, cols], AF.Sqrt, scale=R_STAR * R_STAR)
                nc.vector.tensor_tensor(sg_sb[:, cols], y_sb[:, cols],
                                        rm[:, cols], op=OP.min)
                # mv *= sigma, broadcast over d in one strided multiply
                mv3 = mv_sb[:, t0 * D:t1 * D].rearrange("p (t d) -> p t d", d=D)
                sg_ap = sg_sb[:, cols]
                sgv = bass.AP(sg_ap.tensor, sg_ap.offset,
                              list(sg_ap.ap) + [[0, D]])
                nc.vector.tensor_tensor(mv3, mv3, sgv, op=OP.mult)
                if publish:
                    nc.sync.dma_start(out=xt_shard[:, t0 * D:t1 * D],
                                      in_=mv_sb[:, t0 * D:t1 * D])

            def readout_block(tiles):
                mm_tiles(tiles, wot_sb, out_sb, 16)

            def allgather():
                tc.strict_bb_all_engine_barrier()
                if sim:
                    nc.sync.dma_start(
                        out=xt_table[0:NL, :].rearrange("(p x) d -> p x d", p=P),
                        in_=xt_shard[:].rearrange("p (x d) -> p x d", d=D))
                else:
                    nc.gpsimd.collective_compute(
                        "AllGather", mybir.AluOpType.bypass, replica_groups=groups,
                        ins=[xt_shard[:, :]], outs=[xt_table[:, :]])
                tc.strict_bb_all_engine_barrier()

            def post_agg_group(g0, g1, last_layer):
                """norms of agg & relu(agg); s' chain; t = nu*s'; A'/rn' (or
                stash t2 for the final tanh)."""
                w = g1 - g0
                cols = slice(g0, g1)
                csl = slice(g0 * D, g1 * D)
                sqb = sqp.tile([P, Q * D], f32, tag="sqb")
                act(sqb[:, :w * D], u_sb[:, csl], AF.Square)
                nc.vector.tensor_reduce(
                    na2_sb[:, cols],
                    sqb[:, :w * D].rearrange("p (t d) -> p t d", d=D),
                    axis=X, op=OP.add)
                act(u_sb[:, csl], u_sb[:, csl], AF.Relu)
                sqb2 = sqp.tile([P, Q * D], f32, tag="sqb")
                act(sqb2[:, :w * D], u_sb[:, csl], AF.Square)
                nc.vector.tensor_reduce(
                    nu2_sb[:, cols],
                    sqb2[:, :w * D].rearrange("p (t d) -> p t d", d=D),
                    axis=X, op=OP.add)
                # s' = min(1, R*/na);  rn = 1/nu;  t = nu*s';  A' = min(t, R*)
                sp = scp.tile([P, Q], f32, tag="sp")
                nc.vector.reciprocal(sp[:, :w], na2_sb[:, cols])
                act(sp[:, :w], sp[:, :w], AF.Sqrt, scale=R_STAR * R_STAR)
                nc.vector.tensor_scalar_min(sp[:, :w], sp[:, :w], 1.0)
                z2 = scp.tile([P, Q], f32, tag="z2")
                nc.vector.tensor_scalar_max(z2[:, :w], nu2_sb[:, cols], 1e-30)
                nc.vector.reciprocal(z2[:, :w], z2[:, :w])
                act(rn_sb[:, cols], z2[:, :w], AF.Sqrt)
                nu = scp.tile([P, Q], f32, tag="nu")
                nc.vector.tensor_tensor(nu[:, :w], nu2_sb[:, cols],
                                        rn_sb[:, cols], op=OP.mult)
                tq = scp.tile([P, Q], f32, tag="tq")
                nc.vector.tensor_tensor(tq[:, :w], nu[:, :w], sp[:, :w],
                                        op=OP.mult)
                if last_layer:
                    nc.vector.tensor_scalar_min(t2_sb[:, cols], tq[:, :w],
                                                MAX_TANH)
                else:
                    nc.vector.tensor_scalar_min(A_sb[:, cols], tq[:, :w],
                                                R_STAR)
                    nc.vector.tensor_tensor(y_sb[:, cols], A_sb[:, cols],
                                            rn_sb[:, cols], op=OP.mult)

            # chain-group boundaries: big groups early (hidden under DMA),
            # small groups at the tail to shrink the exposed serial end
            GROUPS = [(0, 8), (8, 16), (16, 24), (24, 28), (28, 30),
                      (30, 31), (31, 32)]
            def gather_layer(layer):
                """aggregate messages; per finished chain-group run the
                post-agg chain and the next stage's linear work."""
                row0_sb = scp.tile([1, D], f32, tag="row0")
                ioff = woff = 0
                gidx = 0
                for t in range(T):
                    K = int(Ks[t])
                    g = gp.tile([P, K * D], f32, tag="G")
                    g3 = g[:].rearrange("p (k d) -> p k d", d=D)
                    nc.gpsimd.dma_gather(
                        g3, xt_table[:, :], idx_sb[:, ioff:ioff + 8 * K],
                        num_idxs=P * K, num_idxs_reg=P * K, elem_size=D,
                        single_packet=False)
                    if t == 0 and not use_wt:
                        # row0 value for the pad-subtract; queued behind the
                        # first gather so it doesn't delay the pipeline start
                        nc.sync.dma_start(out=row0_sb[:], in_=xt_table[0:1, :])
                    if use_wt:
                        wt_ap = wt_sb[:, woff:woff + K]
                        wv = bass.AP(wt_ap.tensor, wt_ap.offset,
                                     list(wt_ap.ap) + [[0, D]])
                        nc.vector.tensor_tensor(g3, g3, wv, op=OP.mult)
                    nc.vector.tensor_reduce(
                        u_sb[:, ts(t)],
                        g[:].rearrange("p (k d) -> p d k", d=D),
                        axis=X, op=OP.add)
                    if not use_wt:
                        corr_ps = psC.tile([P, D], f32, tag="corr")
                        nc.tensor.matmul(
                            out=corr_ps[:], lhsT=pc_sb[0:1, t * P:(t + 1) * P],
                            rhs=row0_sb[0:1, :], start=True, stop=True)
                        nc.vector.tensor_tensor(u_sb[:, ts(t)], u_sb[:, ts(t)],
                                                corr_ps[:], op=OP.subtract)
                    ioff += 8 * K
                    woff += K
                    if t + 1 == GROUPS[gidx][1]:
                        g0, g1 = GROUPS[gidx]
                        gidx += 1
                        post_agg_group(g0, g1, last_layer=(layer == 1))
                        if layer == 0:
                            linear_block(w1t_sb, slice(g0, g1))
                        else:
                            readout_block(slice(g0, g1))

            # ---- layer 1 linear + publish -----------------------------------
            for q in range(NQ):
                intro_quarter(q)
                linear_block(w0t_sb, slice(q * Q, (q + 1) * Q))
            allgather()
            gather_layer(0)
            allgather()
            gather_layer(1)

            # ---- tail: single Tanh, final scale broadcast, store ------------
            th = scp.tile([P, T], f32, tag="th")
            act(th[:], t2_sb[:], AF.Tanh)
            nc.vector.tensor_scalar_min(th[:], th[:], MAXN)
            nc.vector.tensor_tensor(sf_sb[:], th[:], rn_sb[:], op=OP.mult)
            o3 = out_sb[:].rearrange("p (t j) -> p t j", j=16)
            sf_ap = sf_sb[:]
            sfv = bass.AP(sf_ap.tensor, sf_ap.offset, list(sf_ap.ap) + [[0, 16]])
            nc.vector.tensor_tensor(o3, o3, sfv, op=OP.mult)
            nc.sync.dma_start(out=out_dram[:], in_=out_sb[:])
    nc.compile()
    return nc


def kernel(h, distances, rows, cols, node_mask, edge_mask,
           W0, b0, W1, b1, W_out, b_out, _trace=False):
    from concourse.bass_utils import run_bass_kernel_spmd

    h = np.asarray(h, dtype=np.float32)
    rows = np.asarray(rows).astype(np.int64)
    cols = np.asarray(cols).astype(np.int64)
    node_mask = np.asarray(node_mask, dtype=np.float32)
    edge_mask = np.asarray(edge_mask, dtype=np.float32)
    assert not np.any(np.asarray(b0)) and not np.any(np.asarray(b1)) and \
        not np.any(np.asarray(b_out)), "nonzero biases unsupported"

    perm, Ks, idx_dev, wt_dev, IDXC, WTC, pc_dev, allones = _build_tables(
        rows, cols, edge_mask, node_mask)

    hp = h[perm].reshape(C, T, P, D).transpose(0, 2, 1, 3).reshape(C, P, T * D)
    w0t = np.ascontiguousarray(np.vstack([np.asarray(W0, np.float32).T] * 2))
    w1t = np.ascontiguousarray(np.vstack([np.asarray(W1, np.float32).T] * 2))
    wot = np.ascontiguousarray(np.vstack([np.asarray(W_out, np.float32).T] * 2))

    nc = _build_program(Ks, IDXC, WTC, use_wt=not allones)
    in_maps = [{
        "h_in": np.ascontiguousarray(hp[c]),
        "idx_in": idx_dev[c],
        "wt_in": wt_dev[c],
        "w0t_in": w0t, "w1t_in": w1t, "wot_in": wot,
        "pc_in": pc_dev[c],
    } for c in range(C)]
    res = run_bass_kernel_spmd(nc, in_maps, list(range(C)), trace=_trace)
    od = np.stack([res.results[c]["out"] for c in range(C)])
    od = od.reshape(C, P, T, 16).transpose(0, 2, 1, 3).reshape(N, 16)
    out = np.empty((N, 16), np.float32)
    out[perm] = od
    if _trace:
        return out, res
    return out


# revision 56
# speedup vs baseline: 1.0266x; 1.0005x over previous
"""HGCN decoder on 8 trn2 NeuronCores.

Strategy: nodes are sorted by in-degree, grouped into 128-node tiles, and the
tiles are dealt round-robin across the 8 cores (graph-parallel by destination
node).  Each core:
  - runs the node-wise hyperbolic math on its 4096 nodes,
  - publishes its tangent-space table shard, AllGathers the full [32768, 64]
    table to DRAM,
  - aggregates messages with `dma_gather` (padded per-tile CSR) followed by a
    strided reduce on the vector engine,
  - finishes with the euclidean readout matmul.

Key algebraic simplifications vs the literal reference (all exact up to fp
rounding, validated against the jax reference):
  - proj-then-logmap0 collapses: artanh(min(tanh(r), 1-eps)) == min(r, R*)
    with R* = artanh(1-eps).  This removes every Ln/artanh evaluation and all
    but one final Tanh, so the scalar chains are pure min/mult/recip/sqrt.
  - positive per-node scales commute through relu and matmuls, so the vector
    state is kept UNSCALED (raw) and only combined per-node scalars are
    carried between stages.  The only materialized scalings are the published
    message table (xt = mv_raw * sigma) and the final readout.
Activation-table note: the whole kernel needs only {Square, Sqrt, Relu, Copy}
(one table) plus a single trailing Tanh (second table) => 2 table loads total.

All graph preprocessing happens host-side in numpy; the device only sees
dense tables.
"""

import numpy as np

N = 32768
E = 1015808
D = 64
C = 8          # cores
NL = N // C    # 4096 nodes per core
P = 128        # partitions / tile
T = NL // P    # 32 tiles per core
Q = 8          # tiles per chain-quarter
NQ = T // Q
MAXN = 1.0 - 4e-3   # PROJ_EPS boundary for c=1
EPS = 1e-15
MAX_TANH = 15.0
R_STAR = float(np.arctanh(np.float64(1.0 - 4e-3)))  # artanh(MAXN) = 3.10635...


def _build_tables(rows, cols, edge_mask, node_mask):
    """Permute nodes by degree, deal tiles round-robin to cores, and build the
    per-core padded gather tables (int16 indices wrapped the way
    InstDMAGatherAnt wants them) plus matching weight tables."""
    deg = np.bincount(rows, minlength=N)
    order = np.argsort(-deg, kind="stable")
    # global tile j -> core j%C, sorted-slot j//C; slots are then re-ordered so
    # the smallest-K slot runs FIRST (cheap desc-gen before the pipeline
    # fills) and the second-smallest LAST (short exposed tail):
    #   slot order = [smallest, biggest, ..., 2nd smallest]
    S = np.empty(T, dtype=np.int64)
    S[0] = T - 1
    S[1:] = np.arange(T - 1)
    Sinv = np.empty(T, dtype=np.int64)
    Sinv[S] = np.arange(T)
    perm = np.empty(N, dtype=np.int64)
    j = np.arange(N) // P                     # global tile of sorted rank r
    c = j % C
    t = Sinv[j // C]
    p = np.arange(N) % P
    perm[c * NL + t * P + p] = order          # perm[g] = original node id
    pos = np.empty(N, dtype=np.int64)
    pos[perm] = np.arange(N)

    # gather-table row id for permuted position g=(c,t,p):
    #   AllGather concatenates per-core [P, T*D] blocks, so
    #   row_id = c*NL + p*T + t
    gg = np.arange(N)
    gc, gr = gg // NL, gg % NL
    gt, gp_ = gr // P, gr % P
    rowid = gc * NL + gp_ * T + gt            # [g] -> table row
    dstpos = pos[rows]
    eorder = np.argsort(dstpos, kind="stable")
    src_sorted = rowid[pos[cols[eorder]]]     # gather table rows, 0..N-1
    w_sorted = edge_mask[eorder, 0].astype(np.float64)
    cnts = np.bincount(dstpos, minlength=N)
    offs = np.zeros(N + 1, dtype=np.int64)
    np.cumsum(cnts, out=offs[1:])

    # per-slot K: max count over the 8 cores' tiles in that slot
    cnts_g = cnts.reshape(C, T, P)
    Ks = np.maximum(cnts_g.max(axis=(0, 2)), 1).astype(np.int64)   # [T]

    IDXC = int(8 * Ks.sum())
    WTC = int(Ks.sum())
    idx_dev = np.zeros((C, P, IDXC), np.int16)
    wt_dev = np.zeros((C, P, WTC), np.float32)
    nm = node_mask[:, 0].astype(np.float64)
    ioff = woff = 0
    ar = None
    for t in range(T):
        K = int(Ks[t])
        if ar is None or ar.shape[1] != K:
            ar = np.arange(K)[None, :]
        for cc in range(C):
            base = cc * NL + t * P
            cn = cnts[base:base + P]
            take = offs[base:base + P][:, None] + ar          # [P, K]
            valid = ar < cn[:, None]
            take_c = np.minimum(take, E - 1)
            nb = np.where(valid, src_sorted[take_c], 0)
            wl = np.where(valid, w_sorted[take_c], 0.0)
            wl = wl * nm[perm[base:base + P]][:, None]
            il = nb.T.reshape(-1)                             # i = g*128+p
            ch = il.reshape(8 * K, 16).T                      # [16, 8K]
            idx_dev[cc, :, ioff:ioff + 8 * K] = np.tile(ch, (8, 1)).astype(np.int16)
            wt_dev[cc, :, woff:woff + K] = wl.astype(np.float32)
        ioff += 8 * K
        woff += K
    # pad counts per (core, slot, partition) for the pad-subtract path
    pc_dev = np.zeros((C, 1, T * P), np.float32)
    for t in range(T):
        K = int(Ks[t])
        for cc in range(C):
            base = cc * NL + t * P
            pc_dev[cc, 0, t * P:(t + 1) * P] = K - cnts[base:base + P]
    allones = bool(np.all(edge_mask == 1.0) and np.all(node_mask == 1.0))
    return perm, Ks, idx_dev, wt_dev, IDXC, WTC, pc_dev, allones


def _build_program(Ks, IDXC, WTC, use_wt=True, sim=False):
    import concourse.bacc as bacc
    import concourse.bass as bass
    import concourse.mybir as mybir
    import concourse.tile as tile
    from concourse import library_config
    from concourse.masks import make_identity

    f32 = mybir.dt.float32
    i16 = mybir.dt.int16
    AF = mybir.ActivationFunctionType
    OP = mybir.AluOpType
    X = mybir.AxisListType.X

    nc = bacc.Bacc("TRN2", target_bir_lowering=False, debug=False,
                   num_devices=1 if sim else C)

    h_in = nc.dram_tensor("h_in", [P, T * D], f32, kind="ExternalInput")
    idx_in = nc.dram_tensor("idx_in", [P, IDXC], i16, kind="ExternalInput")
    wt_in = nc.dram_tensor("wt_in", [P, WTC], f32, kind="ExternalInput")
    w0t_in = nc.dram_tensor("w0t_in", [2 * D, D], f32, kind="ExternalInput")
    w1t_in = nc.dram_tensor("w1t_in", [2 * D, D], f32, kind="ExternalInput")
    wot_in = nc.dram_tensor("wot_in", [2 * D, 16], f32, kind="ExternalInput")
    pc_in = nc.dram_tensor("pc_in", [1, T * P], f32, kind="ExternalInput")
    out_dram = nc.dram_tensor("out", [P, T * 16], f32, kind="ExternalOutput")
    xt_shard = nc.dram_tensor("xt_shard", [P, T * D], f32)
    xt_table = nc.dram_tensor("xt_table", [N, D], f32, addr_space="Shared")
    groups = [list(range(C))]

    with tile.TileContext(nc) as tc:
        nc.gpsimd.load_library(library_config.mlp)
        import contextlib
        ctx = contextlib.ExitStack()
        with ctx:
            const = ctx.enter_context(tc.tile_pool(name="const", bufs=1))
            sqp = ctx.enter_context(tc.tile_pool(name="sq", bufs=2))
            xtp = ctx.enter_context(tc.tile_pool(name="xtp", bufs=3))
            gp = ctx.enter_context(tc.tile_pool(name="gp", bufs=4))
            scp = ctx.enter_context(tc.tile_pool(name="scp", bufs=3))
            psT = ctx.enter_context(tc.tile_pool(name="psT", bufs=2, space="PSUM"))
            psM = ctx.enter_context(tc.tile_pool(name="psM", bufs=2, space="PSUM"))
            psC = ctx.enter_context(tc.tile_pool(name="psC", bufs=2, space="PSUM"))

            ident = const.tile([P, P], f32)
            make_identity(nc, ident[:])
            # weights duplicated across both partition halves so matmuls can
            # source lhsT from either half of a paired transpose
            w0t_sb = const.tile([2 * D, D], f32)
            w1t_sb = const.tile([2 * D, D], f32)
            wot_sb = const.tile([2 * D, 16], f32)
            idx_sb = const.tile([P, IDXC], i16)
            pc_sb = const.tile([1, T * P], f32)
            if use_wt:
                wt_sb = const.tile([P, WTC], f32)

            u_sb = const.tile([P, T * D], f32)      # raw vector state / agg
            mv_sb = const.tile([P, T * D], f32)     # W@u then scaled messages
            out_sb = const.tile([P, T * 16], f32)
            # long-lived per-node scalars, [P, T] column t = tile t
            A_sb = const.tile([P, T], f32)    # artanh(||x||) of current state
            rn_sb = const.tile([P, T], f32)   # 1 / ||u_raw||
            y_sb = const.tile([P, T], f32)    # A * rn (sigma chain operand)
            t2_sb = const.tile([P, T], f32)   # final tanh argument (layer 2)
            na2_sb = const.tile([P, T], f32)
            nu2_sb = const.tile([P, T], f32)
            mx2_sb = const.tile([P, T], f32)
            sg_sb = const.tile([P, T], f32)   # publish scale sigma per tile
            sf_sb = const.tile([P, T], f32)   # final readout scale

            # load order: h quarter 0 + W0 first (intro/linear need them), the
            # gather metadata afterwards so it streams under the intro compute
            QW = Q * D
            nc.sync.dma_start(out=u_sb[:, 0:QW], in_=h_in[:, 0:QW])
            nc.sync.dma_start(out=w0t_sb[:], in_=w0t_in[:])
            for q in range(1, NQ):
                nc.sync.dma_start(out=u_sb[:, q * QW:(q + 1) * QW],
                                  in_=h_in[:, q * QW:(q + 1) * QW])
            nc.sync.dma_start(out=w1t_sb[:], in_=w1t_in[:])
            nc.sync.dma_start(out=wot_sb[:], in_=wot_in[:])
            nc.sync.dma_start(out=pc_sb[:], in_=pc_in[:])
            nc.sync.dma_start(out=idx_sb[:], in_=idx_in[:])
            if use_wt:
                nc.sync.dma_start(out=wt_sb[:], in_=wt_in[:])

            def ts(t, w=D):
                return slice(t * w, (t + 1) * w)

            def act(out, in_, f, **kw):
                nc.scalar.activation(out, in_, f, **kw)

            def intro_quarter(q):
                """A = min(||h||, R*), rn = 1/||h||, y = A*rn for quarter q."""
                cols = ts(q, Q)
                n02 = scp.tile([P, Q], f32, tag="n02")
                sqb = sqp.tile([P, Q * D], f32, tag="sqb")
                act(sqb[:], u_sb[:, ts(q, Q * D)], AF.Square)
                nc.vector.tensor_reduce(
                    n02[:], sqb[:].rearrange("p (t d) -> p t d", d=D),
                    axis=X, op=OP.add)
                z = scp.tile([P, Q], f32, tag="z0")
                nc.vector.reciprocal(z[:], n02[:])
                act(rn_sb[:, cols], z[:], AF.Sqrt)          # 1/n0
                n0 = scp.tile([P, Q], f32, tag="n0")
                nc.vector.tensor_tensor(n0[:], n02[:], rn_sb[:, cols],
                                        op=OP.mult)         # n0
                nc.vector.tensor_scalar_min(A_sb[:, cols], n0[:], R_STAR)
                nc.vector.tensor_tensor(y_sb[:, cols], A_sb[:, cols],
                                        rn_sb[:, cols], op=OP.mult)

            def mm_tiles(tiles, w_sb, dst_sb, ow):
                """transpose+matmul u[tiles] @ W^T -> dst_sb (free width ow),
                batching up to 4 matmul outputs per PSUM tile/copy."""
                t0, t1 = tiles.start, tiles.stop
                for tt in range(t0, t1):
                    j = tt - t0
                    if j % 2 == 0:
                        np2 = min(2, t1 - tt)
                        ps = psT.tile([P, P], f32, tag="xT2")
                        nc.tensor.transpose(
                            out=ps[0:np2 * D, :],
                            in_=u_sb[:, tt * D:(tt + np2) * D],
                            identity=ident[:])
                        xT2 = xtp.tile([P, P], f32, tag="xT2sb")
                        nc.vector.tensor_copy(xT2[0:np2 * D, :],
                                              ps[0:np2 * D, :])
                    h2 = j % 2
                    # NOTE: matmul PSUM destinations must be whole pool tiles;
                    # column-sliced outputs crash the device (works in sim).
                    pm = psM.tile([P, ow], f32, tag="mv")
                    nc.tensor.matmul(out=pm[:],
                                     lhsT=xT2[h2 * D:(h2 + 1) * D, :],
                                     rhs=w_sb[h2 * D:(h2 + 1) * D, :],
                                     start=True, stop=True)
                    act(dst_sb[:, tt * ow:(tt + 1) * ow], pm[:], AF.Copy)

            def linear_block(w_sb, tiles, publish=True):
                """mv_raw = u @ W^T for `tiles`; sigma = min(y, R*/mraw);
                mv *= sigma; publish.  (y = A*rn precomputed per node.)"""
                t0, t1 = tiles.start, tiles.stop
                nt = t1 - t0
                mm_tiles(tiles, w_sb, mv_sb, D)
                # batched ||mv||^2 then the fused sigma chain
                for j in range((nt + Q - 1) // Q):
                    c0 = t0 + j * Q
                    c1 = min(c0 + Q, t1)
                    sqb = sqp.tile([P, Q * D], f32, tag="sqb")
                    act(sqb[:, :(c1 - c0) * D], mv_sb[:, c0 * D:c1 * D], AF.Square)
                    nc.vector.tensor_reduce(
                        mx2_sb[:, c0:c1],
                        sqb[:, :(c1 - c0) * D].rearrange("p (t d) -> p t d", d=D),
                        axis=X, op=OP.add)
                cols = slice(t0, t1)
                rm = scp.tile([P, T], f32, tag="rm")
                nc.vector.reciprocal(rm[:, cols], mx2_sb[:, cols])
                act(rm[:, cols], rm[:, cols], AF.Sqrt, scale=R_STAR * R_STAR)
                nc.vector.tensor_tensor(sg_sb[:, cols], y_sb[:, cols],
                                        rm[:, cols], op=OP.min)
                # mv *= sigma, broadcast over d in one strided multiply
                mv3 = mv_sb[:, t0 * D:t1 * D].rearrange("p (t d) -> p t d", d=D)
                sg_ap = sg_sb[:, cols]
                sgv = bass.AP(sg_ap.tensor, sg_ap.offset,
                              list(sg_ap.ap) + [[0, D]])
                nc.vector.tensor_tensor(mv3, mv3, sgv, op=OP.mult)
                if publish:
                    nc.sync.dma_start(out=xt_shard[:, t0 * D:t1 * D],
                                      in_=mv_sb[:, t0 * D:t1 * D])

            def readout_block(tiles):
                mm_tiles(tiles, wot_sb, out_sb, 16)

            def allgather():
                tc.strict_bb_all_engine_barrier()
                if sim:
                    nc.sync.dma_start(
                        out=xt_table[0:NL, :].rearrange("(p x) d -> p x d", p=P),
                        in_=xt_shard[:].rearrange("p (x d) -> p x d", d=D))
                else:
                    nc.gpsimd.collective_compute(
                        "AllGather", mybir.AluOpType.bypass, replica_groups=groups,
                        ins=[xt_shard[:, :]], outs=[xt_table[:, :]])

            def post_agg_group(g0, g1, last_layer):
                """norms of agg & relu(agg); s' chain; t = nu*s'; A'/rn' (or
                stash t2 for the final tanh)."""
                w = g1 - g0
                cols = slice(g0, g1)
                csl = slice(g0 * D, g1 * D)
                sqb = sqp.tile([P, Q * D], f32, tag="sqb")
                act(sqb[:, :w * D], u_sb[:, csl], AF.Square)
                nc.vector.tensor_reduce(
                    na2_sb[:, cols],
                    sqb[:, :w * D].rearrange("p (t d) -> p t d", d=D),
                    axis=X, op=OP.add)
                act(u_sb[:, csl], u_sb[:, csl], AF.Relu)
                sqb2 = sqp.tile([P, Q * D], f32, tag="sqb")
                act(sqb2[:, :w * D], u_sb[:, csl], AF.Square)
                nc.vector.tensor_reduce(
                    nu2_sb[:, cols],
                    sqb2[:, :w * D].rearrange("p (t d) -> p t d", d=D),
                    axis=X, op=OP.add)
                # s' = min(1, R*/na);  rn = 1/nu;  t = nu*s';  A' = min(t, R*)
                sp = scp.tile([P, Q], f32, tag="sp")
                nc.vector.reciprocal(sp[:, :w], na2_sb[:, cols])
                act(sp[:, :w], sp[:, :w], AF.Sqrt, scale=R_STAR * R_STAR)
                nc.vector.tensor_scalar_min(sp[:, :w], sp[:, :w], 1.0)
                z2 = scp.tile([P, Q], f32, tag="z2")
                nc.vector.tensor_scalar_max(z2[:, :w], nu2_sb[:, cols], 1e-30)
                nc.vector.reciprocal(z2[:, :w], z2[:, :w])
                act(rn_sb[:, cols], z2[:, :w], AF.Sqrt)
                nu = scp.tile([P, Q], f32, tag="nu")
                nc.vector.tensor_tensor(nu[:, :w], nu2_sb[:, cols],
                                        rn_sb[:, cols], op=OP.mult)
                tq = scp.tile([P, Q], f32, tag="tq")
                nc.vector.tensor_tensor(tq[:, :w], nu[:, :w], sp[:, :w],
                                        op=OP.mult)
                if last_layer:
                    nc.vector.tensor_scalar_min(t2_sb[:, cols], tq[:, :w],
                                                MAX_TANH)
                else:
                    nc.vector.tensor_scalar_min(A_sb[:, cols], tq[:, :w],
                                                R_STAR)
                    nc.vector.tensor_tensor(y_sb[:, cols], A_sb[:, cols],
                                            rn_sb[:, cols], op=OP.mult)

            # chain-group boundaries: big groups early (hidden under DMA),
            # small groups at the tail to shrink the exposed serial end
            GROUPS = [(0, 8), (8, 16), (16, 24), (24, 28), (28, 30),
                      (30, 31), (31, 32)]
            def gather_layer(layer):
                """aggregate messages; per finished chain-group run the
                post-agg chain and the next stage's linear work."""
                row0_sb = scp.tile([1, D], f32, tag="row0")
                ioff = woff = 0
                gidx = 0
                for t in range(T):
                    K = int(Ks[t])
                    g = gp.tile([P, K * D], f32, tag="G")
                    g3 = g[:].rearrange("p (k d) -> p k d", d=D)
                    nc.gpsimd.dma_gather(
                        g3, xt_table[:, :], idx_sb[:, ioff:ioff + 8 * K],
                        num_idxs=P * K, num_idxs_reg=P * K, elem_size=D,
                        single_packet=False)
                    if t == 0 and not use_wt:
                        # row0 value for the pad-subtract; queued behind the
                        # first gather so it doesn't delay the pipeline start
                        nc.sync.dma_start(out=row0_sb[:], in_=xt_table[0:1, :])
                    if use_wt:
                        wt_ap = wt_sb[:, woff:woff + K]
                        wv = bass.AP(wt_ap.tensor, wt_ap.offset,
                                     list(wt_ap.ap) + [[0, D]])
                        nc.vector.tensor_tensor(g3, g3, wv, op=OP.mult)
                    nc.vector.tensor_reduce(
                        u_sb[:, ts(t)],
                        g[:].rearrange("p (k d) -> p d k", d=D),
                        axis=X, op=OP.add)
                    if not use_wt:
                        corr_ps = psC.tile([P, D], f32, tag="corr")
                        nc.tensor.matmul(
                            out=corr_ps[:], lhsT=pc_sb[0:1, t * P:(t + 1) * P],
                            rhs=row0_sb[0:1, :], start=True, stop=True)
                        nc.vector.tensor_tensor(u_sb[:, ts(t)], u_sb[:, ts(t)],
                                                corr_ps[:], op=OP.subtract)
                    ioff += 8 * K
                    woff += K
                    if t + 1 == GROUPS[gidx][1]:
                        g0, g1 = GROUPS[gidx]
                        gidx += 1
                        post_agg_group(g0, g1, last_layer=(layer == 1))
                        if layer == 0:
                            linear_block(w1t_sb, slice(g0, g1))
                        else:
                            readout_block(slice(g0, g1))

            # ---- layer 1 linear + publish -----------------------------------
            for q in range(NQ):
                intro_quarter(q)
                linear_block(w0t_sb, slice(q * Q, (q + 1) * Q))
            allgather()
            gather_layer(0)
            allgather()
            gather_layer(1)

            # ---- tail: single Tanh, final scale broadcast, store ------------
            th = scp.tile([P, T], f32, tag="th")
            act(th[:], t2_sb[:], AF.Tanh)
            nc.vector.tensor_scalar_min(th[:], th[:], MAXN)
            nc.vector.tensor_tensor(sf_sb[:], th[:], rn_sb[:], op=OP.mult)
            o3 = out_sb[:].rearrange("p (t j) -> p t j", j=16)
            sf_ap = sf_sb[:]
            sfv = bass.AP(sf_ap.tensor, sf_ap.offset, list(sf_ap.ap) + [[0, 16]])
            nc.vector.tensor_tensor(o3, o3, sfv, op=OP.mult)
            nc.sync.dma_start(out=out_dram[:], in_=out_sb[:])
    nc.compile()
    return nc


def kernel(h, distances, rows, cols, node_mask, edge_mask,
           W0, b0, W1, b1, W_out, b_out, _trace=False):
    from concourse.bass_utils import run_bass_kernel_spmd

    h = np.asarray(h, dtype=np.float32)
    rows = np.asarray(rows).astype(np.int64)
    cols = np.asarray(cols).astype(np.int64)
    node_mask = np.asarray(node_mask, dtype=np.float32)
    edge_mask = np.asarray(edge_mask, dtype=np.float32)
    assert not np.any(np.asarray(b0)) and not np.any(np.asarray(b1)) and \
        not np.any(np.asarray(b_out)), "nonzero biases unsupported"

    perm, Ks, idx_dev, wt_dev, IDXC, WTC, pc_dev, allones = _build_tables(
        rows, cols, edge_mask, node_mask)

    hp = h[perm].reshape(C, T, P, D).transpose(0, 2, 1, 3).reshape(C, P, T * D)
    w0t = np.ascontiguousarray(np.vstack([np.asarray(W0, np.float32).T] * 2))
    w1t = np.ascontiguousarray(np.vstack([np.asarray(W1, np.float32).T] * 2))
    wot = np.ascontiguousarray(np.vstack([np.asarray(W_out, np.float32).T] * 2))

    nc = _build_program(Ks, IDXC, WTC, use_wt=not allones)
    in_maps = [{
        "h_in": np.ascontiguousarray(hp[c]),
        "idx_in": idx_dev[c],
        "wt_in": wt_dev[c],
        "w0t_in": w0t, "w1t_in": w1t, "wot_in": wot,
        "pc_in": pc_dev[c],
    } for c in range(C)]
    res = run_bass_kernel_spmd(nc, in_maps, list(range(C)), trace=_trace)
    od = np.stack([res.results[c]["out"] for c in range(C)])
    od = od.reshape(C, P, T, 16).transpose(0, 2, 1, 3).reshape(N, 16)
    out = np.empty((N, 16), np.float32)
    out[perm] = od
    if _trace:
        return out, res
    return out


# revision 57
# speedup vs baseline: 1.0277x; 1.0011x over previous
"""HGCN decoder on 8 trn2 NeuronCores.

Strategy: nodes are sorted by in-degree, grouped into 128-node tiles, and the
tiles are dealt round-robin across the 8 cores (graph-parallel by destination
node).  Each core:
  - runs the node-wise hyperbolic math on its 4096 nodes,
  - publishes its tangent-space table shard, AllGathers the full [32768, 64]
    table to DRAM,
  - aggregates messages with `dma_gather` (padded per-tile CSR) followed by a
    strided reduce on the vector engine,
  - finishes with the euclidean readout matmul.

Key algebraic simplifications vs the literal reference (all exact up to fp
rounding, validated against the jax reference):
  - proj-then-logmap0 collapses: artanh(min(tanh(r), 1-eps)) == min(r, R*)
    with R* = artanh(1-eps).  This removes every Ln/artanh evaluation and all
    but one final Tanh, so the scalar chains are pure min/mult/recip/sqrt.
  - positive per-node scales commute through relu and matmuls, so the vector
    state is kept UNSCALED (raw) and only combined per-node scalars are
    carried between stages.  The only materialized scalings are the published
    message table (xt = mv_raw * sigma) and the final readout.
Activation-table note: the whole kernel needs only {Square, Sqrt, Relu, Copy}
(one table) plus a single trailing Tanh (second table) => 2 table loads total.

All graph preprocessing happens host-side in numpy; the device only sees
dense tables.
"""

import numpy as np

N = 32768
E = 1015808
D = 64
C = 8          # cores
NL = N // C    # 4096 nodes per core
P = 128        # partitions / tile
T = NL // P    # 32 tiles per core
Q = 8          # tiles per chain-quarter
NQ = T // Q
MAXN = 1.0 - 4e-3   # PROJ_EPS boundary for c=1
EPS = 1e-15
MAX_TANH = 15.0
R_STAR = float(np.arctanh(np.float64(1.0 - 4e-3)))  # artanh(MAXN) = 3.10635...


def _build_tables(rows, cols, edge_mask, node_mask):
    """Permute nodes by degree, deal tiles round-robin to cores, and build the
    per-core padded gather tables (int16 indices wrapped the way
    InstDMAGatherAnt wants them) plus matching weight tables."""
    deg = np.bincount(rows, minlength=N)
    order = np.argsort(-deg, kind="stable")
    # global tile j -> core j%C, sorted-slot j//C; slots are then re-ordered so
    # the smallest-K slot runs FIRST (cheap desc-gen before the pipeline
    # fills) and the second-smallest LAST (short exposed tail):
    #   slot order = [smallest, biggest, ..., 2nd smallest]
    S = np.empty(T, dtype=np.int64)
    S[0] = T - 1
    S[1:] = np.arange(T - 1)
    Sinv = np.empty(T, dtype=np.int64)
    Sinv[S] = np.arange(T)
    perm = np.empty(N, dtype=np.int64)
    j = np.arange(N) // P                     # global tile of sorted rank r
    c = j % C
    t = Sinv[j // C]
    p = np.arange(N) % P
    perm[c * NL + t * P + p] = order          # perm[g] = original node id
    pos = np.empty(N, dtype=np.int64)
    pos[perm] = np.arange(N)

    # gather-table row id for permuted position g=(c,t,p):
    #   AllGather concatenates per-core [P, T*D] blocks, so
    #   row_id = c*NL + p*T + t
    gg = np.arange(N)
    gc, gr = gg // NL, gg % NL
    gt, gp_ = gr // P, gr % P
    rowid = gc * NL + gp_ * T + gt            # [g] -> table row
    dstpos = pos[rows]
    eorder = np.argsort(dstpos, kind="stable")
    src_sorted = rowid[pos[cols[eorder]]]     # gather table rows, 0..N-1
    w_sorted = edge_mask[eorder, 0].astype(np.float64)
    cnts = np.bincount(dstpos, minlength=N)
    offs = np.zeros(N + 1, dtype=np.int64)
    np.cumsum(cnts, out=offs[1:])

    # per-slot K: max count over the 8 cores' tiles in that slot
    cnts_g = cnts.reshape(C, T, P)
    Ks = np.maximum(cnts_g.max(axis=(0, 2)), 1).astype(np.int64)   # [T]

    IDXC = int(8 * Ks.sum())
    WTC = int(Ks.sum())
    idx_dev = np.zeros((C, P, IDXC), np.int16)
    wt_dev = np.zeros((C, P, WTC), np.float32)
    nm = node_mask[:, 0].astype(np.float64)
    ioff = woff = 0
    ar = None
    for t in range(T):
        K = int(Ks[t])
        if ar is None or ar.shape[1] != K:
            ar = np.arange(K)[None, :]
        for cc in range(C):
            base = cc * NL + t * P
            cn = cnts[base:base + P]
            take = offs[base:base + P][:, None] + ar          # [P, K]
            valid = ar < cn[:, None]
            take_c = np.minimum(take, E - 1)
            nb = np.where(valid, src_sorted[take_c], 0)
            wl = np.where(valid, w_sorted[take_c], 0.0)
            wl = wl * nm[perm[base:base + P]][:, None]
            il = nb.T.reshape(-1)                             # i = g*128+p
            ch = il.reshape(8 * K, 16).T                      # [16, 8K]
            idx_dev[cc, :, ioff:ioff + 8 * K] = np.tile(ch, (8, 1)).astype(np.int16)
            wt_dev[cc, :, woff:woff + K] = wl.astype(np.float32)
        ioff += 8 * K
        woff += K
    # pad counts per (core, slot, partition) for the pad-subtract path
    pc_dev = np.zeros((C, 1, T * P), np.float32)
    for t in range(T):
        K = int(Ks[t])
        for cc in range(C):
            base = cc * NL + t * P
            pc_dev[cc, 0, t * P:(t + 1) * P] = K - cnts[base:base + P]
    allones = bool(np.all(edge_mask == 1.0) and np.all(node_mask == 1.0))
    return perm, Ks, idx_dev, wt_dev, IDXC, WTC, pc_dev, allones


def _build_program(Ks, IDXC, WTC, use_wt=True, sim=False):
    import concourse.bacc as bacc
    import concourse.bass as bass
    import concourse.mybir as mybir
    import concourse.tile as tile
    from concourse import library_config
    from concourse.masks import make_identity

    f32 = mybir.dt.float32
    i16 = mybir.dt.int16
    AF = mybir.ActivationFunctionType
    OP = mybir.AluOpType
    X = mybir.AxisListType.X

    nc = bacc.Bacc("TRN2", target_bir_lowering=False, debug=False,
                   num_devices=1 if sim else C)

    h_in = nc.dram_tensor("h_in", [P, T * D], f32, kind="ExternalInput")
    idx_in = nc.dram_tensor("idx_in", [P, IDXC], i16, kind="ExternalInput")
    wt_in = nc.dram_tensor("wt_in", [P, WTC], f32, kind="ExternalInput")
    w0t_in = nc.dram_tensor("w0t_in", [2 * D, D], f32, kind="ExternalInput")
    w1t_in = nc.dram_tensor("w1t_in", [2 * D, D], f32, kind="ExternalInput")
    wot_in = nc.dram_tensor("wot_in", [2 * D, 16], f32, kind="ExternalInput")
    pc_in = nc.dram_tensor("pc_in", [1, T * P], f32, kind="ExternalInput")
    out_dram = nc.dram_tensor("out", [P, T * 16], f32, kind="ExternalOutput")
    xt_shard = nc.dram_tensor("xt_shard", [P, T * D], f32)
    xt_table = nc.dram_tensor("xt_table", [N, D], f32, addr_space="Shared")
    groups = [list(range(C))]

    with tile.TileContext(nc) as tc:
        nc.gpsimd.load_library(library_config.mlp)
        import contextlib
        ctx = contextlib.ExitStack()
        with ctx:
            const = ctx.enter_context(tc.tile_pool(name="const", bufs=1))
            sqp = ctx.enter_context(tc.tile_pool(name="sq", bufs=2))
            xtp = ctx.enter_context(tc.tile_pool(name="xtp", bufs=3))
            gp = ctx.enter_context(tc.tile_pool(name="gp", bufs=4))
            scp = ctx.enter_context(tc.tile_pool(name="scp", bufs=3))
            psT = ctx.enter_context(tc.tile_pool(name="psT", bufs=2, space="PSUM"))
            psM = ctx.enter_context(tc.tile_pool(name="psM", bufs=2, space="PSUM"))
            psC = ctx.enter_context(tc.tile_pool(name="psC", bufs=2, space="PSUM"))

            ident = const.tile([P, P], f32)
            make_identity(nc, ident[:])
            # weights duplicated across both partition halves so matmuls can
            # source lhsT from either half of a paired transpose
            w0t_sb = const.tile([2 * D, D], f32)
            w1t_sb = const.tile([2 * D, D], f32)
            wot_sb = const.tile([2 * D, 16], f32)
            idx_sb = const.tile([P, IDXC], i16)
            pc_sb = const.tile([1, T * P], f32)
            if use_wt:
                wt_sb = const.tile([P, WTC], f32)

            u_sb = const.tile([P, T * D], f32)      # raw vector state / agg
            mv_sb = const.tile([P, T * D], f32)     # W@u then scaled messages
            out_sb = const.tile([P, T * 16], f32)
            # long-lived per-node scalars, [P, T] column t = tile t
            A_sb = const.tile([P, T], f32)    # artanh(||x||) of current state
            rn_sb = const.tile([P, T], f32)   # 1 / ||u_raw||
            y_sb = const.tile([P, T], f32)    # A * rn (sigma chain operand)
            t2_sb = const.tile([P, T], f32)   # final tanh argument (layer 2)
            na2_sb = const.tile([P, T], f32)
            nu2_sb = const.tile([P, T], f32)
            mx2_sb = const.tile([P, T], f32)
            sg_sb = const.tile([P, T], f32)   # publish scale sigma per tile
            sf_sb = const.tile([P, T], f32)   # final readout scale

            # load order: h quarter 0 + W0 first (intro/linear need them), the
            # gather metadata afterwards so it streams under the intro compute
            QW = Q * D
            nc.sync.dma_start(out=u_sb[:, 0:QW], in_=h_in[:, 0:QW])
            nc.sync.dma_start(out=w0t_sb[:], in_=w0t_in[:])
            for q in range(1, NQ):
                nc.sync.dma_start(out=u_sb[:, q * QW:(q + 1) * QW],
                                  in_=h_in[:, q * QW:(q + 1) * QW])
            nc.sync.dma_start(out=w1t_sb[:], in_=w1t_in[:])
            nc.sync.dma_start(out=wot_sb[:], in_=wot_in[:])
            nc.sync.dma_start(out=pc_sb[:], in_=pc_in[:])
            nc.sync.dma_start(out=idx_sb[:], in_=idx_in[:])
            if use_wt:
                nc.sync.dma_start(out=wt_sb[:], in_=wt_in[:])

            def ts(t, w=D):
                return slice(t * w, (t + 1) * w)

            def act(out, in_, f, **kw):
                nc.scalar.activation(out, in_, f, **kw)

            def intro_quarter(q):
                """A = min(||h||, R*), rn = 1/||h||, y = A*rn for quarter q."""
                cols = ts(q, Q)
                n02 = scp.tile([P, Q], f32, tag="n02")
                sqb = sqp.tile([P, Q * D], f32, tag="sqb")
                act(sqb[:], u_sb[:, ts(q, Q * D)], AF.Square)
                nc.vector.tensor_reduce(
                    n02[:], sqb[:].rearrange("p (t d) -> p t d", d=D),
                    axis=X, op=OP.add)
                z = scp.tile([P, Q], f32, tag="z0")
                nc.vector.reciprocal(z[:], n02[:])
                act(rn_sb[:, cols], z[:], AF.Sqrt)          # 1/n0
                n0 = scp.tile([P, Q], f32, tag="n0")
                nc.vector.tensor_tensor(n0[:], n02[:], rn_sb[:, cols],
                                        op=OP.mult)         # n0
                nc.vector.tensor_scalar_min(A_sb[:, cols], n0[:], R_STAR)
                nc.vector.tensor_tensor(y_sb[:, cols], A_sb[:, cols],
                                        rn_sb[:, cols], op=OP.mult)

            def mm_tiles(tiles, w_sb, dst_sb, ow):
                """transpose+matmul u[tiles] @ W^T -> dst_sb (free width ow),
                batching up to 4 matmul outputs per PSUM tile/copy."""
                t0, t1 = tiles.start, tiles.stop
                for tt in range(t0, t1):
                    j = tt - t0
                    if j % 2 == 0:
                        np2 = min(2, t1 - tt)
                        ps = psT.tile([P, P], f32, tag="xT2")
                        nc.tensor.transpose(
                            out=ps[0:np2 * D, :],
                            in_=u_sb[:, tt * D:(tt + np2) * D],
                            identity=ident[:])
                        xT2 = xtp.tile([P, P], f32, tag="xT2sb")
                        nc.vector.tensor_copy(xT2[0:np2 * D, :],
                                              ps[0:np2 * D, :])
                    h2 = j % 2
                    # NOTE: matmul PSUM destinations must be whole pool tiles;
                    # column-sliced outputs crash the device (works in sim).
                    pm = psM.tile([P, ow], f32, tag="mv")
                    nc.tensor.matmul(out=pm[:],
                                     lhsT=xT2[h2 * D:(h2 + 1) * D, :],
                                     rhs=w_sb[h2 * D:(h2 + 1) * D, :],
                                     start=True, stop=True)
                    act(dst_sb[:, tt * ow:(tt + 1) * ow], pm[:], AF.Copy)

            def linear_block(w_sb, tiles, publish=True):
                """mv_raw = u @ W^T for `tiles`; sigma = min(y, R*/mraw);
                mv *= sigma; publish.  (y = A*rn precomputed per node.)"""
                t0, t1 = tiles.start, tiles.stop
                nt = t1 - t0
                mm_tiles(tiles, w_sb, mv_sb, D)
                # batched ||mv||^2 then the fused sigma chain
                for j in range((nt + Q - 1) // Q):
                    c0 = t0 + j * Q
                    c1 = min(c0 + Q, t1)
                    sqb = sqp.tile([P, Q * D], f32, tag="sqb")
                    act(sqb[:, :(c1 - c0) * D], mv_sb[:, c0 * D:c1 * D], AF.Square)
                    nc.vector.tensor_reduce(
                        mx2_sb[:, c0:c1],
                        sqb[:, :(c1 - c0) * D].rearrange("p (t d) -> p t d", d=D),
                        axis=X, op=OP.add)
                cols = slice(t0, t1)
                rm = scp.tile([P, T], f32, tag="rm")
                nc.vector.reciprocal(rm[:, cols], mx2_sb[:, cols])
                act(rm[:, cols], rm[:, cols], AF.Sqrt, scale=R_STAR * R_STAR)
                nc.vector.tensor_tensor(sg_sb[:, cols], y_sb[:, cols],
                                        rm[:, cols], op=OP.min)
                # mv *= sigma, broadcast over d in one strided multiply
                mv3 = mv_sb[:, t0 * D:t1 * D].rearrange("p (t d) -> p t d", d=D)
                sg_ap = sg_sb[:, cols]
                sgv = bass.AP(sg_ap.tensor, sg_ap.offset,
                              list(sg_ap.ap) + [[0, D]])
                nc.vector.tensor_tensor(mv3, mv3, sgv, op=OP.mult)
                if publish:
                    nc.sync.dma_start(out=xt_shard[:, t0 * D:t1 * D],
                                      in_=mv_sb[:, t0 * D:t1 * D])

            def readout_block(tiles):
                mm_tiles(tiles, wot_sb, out_sb, 16)

            def allgather():
                if sim:
                    nc.sync.dma_start(
                        out=xt_table[0:NL, :].rearrange("(p x) d -> p x d", p=P),
                        in_=xt_shard[:].rearrange("p (x d) -> p x d", d=D))
                else:
                    nc.gpsimd.collective_compute(
                        "AllGather", mybir.AluOpType.bypass, replica_groups=groups,
                        ins=[xt_shard[:, :]], outs=[xt_table[:, :]])

            def post_agg_group(g0, g1, last_layer):
                """norms of agg & relu(agg); s' chain; t = nu*s'; A'/rn' (or
                stash t2 for the final tanh)."""
                w = g1 - g0
                cols = slice(g0, g1)
                csl = slice(g0 * D, g1 * D)
                sqb = sqp.tile([P, Q * D], f32, tag="sqb")
                act(sqb[:, :w * D], u_sb[:, csl], AF.Square)
                nc.vector.tensor_reduce(
                    na2_sb[:, cols],
                    sqb[:, :w * D].rearrange("p (t d) -> p t d", d=D),
                    axis=X, op=OP.add)
                act(u_sb[:, csl], u_sb[:, csl], AF.Relu)
                sqb2 = sqp.tile([P, Q * D], f32, tag="sqb")
                act(sqb2[:, :w * D], u_sb[:, csl], AF.Square)
                nc.vector.tensor_reduce(
                    nu2_sb[:, cols],
                    sqb2[:, :w * D].rearrange("p (t d) -> p t d", d=D),
                    axis=X, op=OP.add)
                # s' = min(1, R*/na);  rn = 1/nu;  t = nu*s';  A' = min(t, R*)
                sp = scp.tile([P, Q], f32, tag="sp")
                nc.vector.reciprocal(sp[:, :w], na2_sb[:, cols])
                act(sp[:, :w], sp[:, :w], AF.Sqrt, scale=R_STAR * R_STAR)
                nc.vector.tensor_scalar_min(sp[:, :w], sp[:, :w], 1.0)
                z2 = scp.tile([P, Q], f32, tag="z2")
                nc.vector.tensor_scalar_max(z2[:, :w], nu2_sb[:, cols], 1e-30)
                nc.vector.reciprocal(z2[:, :w], z2[:, :w])
                act(rn_sb[:, cols], z2[:, :w], AF.Sqrt)
                nu = scp.tile([P, Q], f32, tag="nu")
                nc.vector.tensor_tensor(nu[:, :w], nu2_sb[:, cols],
                                        rn_sb[:, cols], op=OP.mult)
                tq = scp.tile([P, Q], f32, tag="tq")
                nc.vector.tensor_tensor(tq[:, :w], nu[:, :w], sp[:, :w],
                                        op=OP.mult)
                if last_layer:
                    nc.vector.tensor_scalar_min(t2_sb[:, cols], tq[:, :w],
                                                MAX_TANH)
                else:
                    nc.vector.tensor_scalar_min(A_sb[:, cols], tq[:, :w],
                                                R_STAR)
                    nc.vector.tensor_tensor(y_sb[:, cols], A_sb[:, cols],
                                            rn_sb[:, cols], op=OP.mult)

            # chain-group boundaries: big groups early (hidden under DMA),
            # small groups at the tail to shrink the exposed serial end
            GROUPS = [(0, 8), (8, 16), (16, 24), (24, 28), (28, 30),
                      (30, 31), (31, 32)]
            def gather_layer(layer):
                """aggregate messages; per finished chain-group run the
                post-agg chain and the next stage's linear work."""
                row0_sb = scp.tile([1, D], f32, tag="row0")
                ioff = woff = 0
                gidx = 0
                for t in range(T):
                    K = int(Ks[t])
                    g = gp.tile([P, K * D], f32, tag="G")
                    g3 = g[:].rearrange("p (k d) -> p k d", d=D)
                    nc.gpsimd.dma_gather(
                        g3, xt_table[:, :], idx_sb[:, ioff:ioff + 8 * K],
                        num_idxs=P * K, num_idxs_reg=P * K, elem_size=D,
                        single_packet=False)
                    if t == 0 and not use_wt:
                        # row0 value for the pad-subtract; queued behind the
                        # first gather so it doesn't delay the pipeline start
                        nc.sync.dma_start(out=row0_sb[:], in_=xt_table[0:1, :])
                    if use_wt:
                        wt_ap = wt_sb[:, woff:woff + K]
                        wv = bass.AP(wt_ap.tensor, wt_ap.offset,
                                     list(wt_ap.ap) + [[0, D]])
                        nc.vector.tensor_tensor(g3, g3, wv, op=OP.mult)
                    nc.vector.tensor_reduce(
                        u_sb[:, ts(t)],
                        g[:].rearrange("p (k d) -> p d k", d=D),
                        axis=X, op=OP.add)
                    if not use_wt:
                        corr_ps = psC.tile([P, D], f32, tag="corr")
                        nc.tensor.matmul(
                            out=corr_ps[:], lhsT=pc_sb[0:1, t * P:(t + 1) * P],
                            rhs=row0_sb[0:1, :], start=True, stop=True)
                        nc.vector.tensor_tensor(u_sb[:, ts(t)], u_sb[:, ts(t)],
                                                corr_ps[:], op=OP.subtract)
                    ioff += 8 * K
                    woff += K
                    if t + 1 == GROUPS[gidx][1]:
                        g0, g1 = GROUPS[gidx]
                        gidx += 1
                        post_agg_group(g0, g1, last_layer=(layer == 1))
                        if layer == 0:
                            linear_block(w1t_sb, slice(g0, g1))
                        else:
                            readout_block(slice(g0, g1))

            # ---- layer 1 linear + publish -----------------------------------
            for q in range(NQ):
                intro_quarter(q)
                linear_block(w0t_sb, slice(q * Q, (q + 1) * Q))
            allgather()
            gather_layer(0)
            allgather()
            gather_layer(1)

            # ---- tail: single Tanh, final scale broadcast, store ------------
            th = scp.tile([P, T], f32, tag="th")
            act(th[:], t2_sb[:], AF.Tanh)
            nc.vector.tensor_scalar_min(th[:], th[:], MAXN)
            nc.vector.tensor_tensor(sf_sb[:], th[:], rn_sb[:], op=OP.mult)
            o3 = out_sb[:].rearrange("p (t j) -> p t j", j=16)
            sf_ap = sf_sb[:]
            sfv = bass.AP(sf_ap.tensor, sf_ap.offset, list(sf_ap.ap) + [[0, 16]])
            nc.vector.tensor_tensor(o3, o3, sfv, op=OP.mult)
            nc.sync.dma_start(out=out_dram[:], in_=out_sb[:])
    nc.compile()
    return nc


def kernel(h, distances, rows, cols, node_mask, edge_mask,
           W0, b0, W1, b1, W_out, b_out, _trace=False):
    from concourse.bass_utils import run_bass_kernel_spmd

    h = np.asarray(h, dtype=np.float32)
    rows = np.asarray(rows).astype(np.int64)
    cols = np.asarray(cols).astype(np.int64)
    node_mask = np.asarray(node_mask, dtype=np.float32)
    edge_mask = np.asarray(edge_mask, dtype=np.float32)
    assert not np.any(np.asarray(b0)) and not np.any(np.asarray(b1)) and \
        not np.any(np.asarray(b_out)), "nonzero biases unsupported"

    perm, Ks, idx_dev, wt_dev, IDXC, WTC, pc_dev, allones = _build_tables(
        rows, cols, edge_mask, node_mask)

    hp = h[perm].reshape(C, T, P, D).transpose(0, 2, 1, 3).reshape(C, P, T * D)
    w0t = np.ascontiguousarray(np.vstack([np.asarray(W0, np.float32).T] * 2))
    w1t = np.ascontiguousarray(np.vstack([np.asarray(W1, np.float32).T] * 2))
    wot = np.ascontiguousarray(np.vstack([np.asarray(W_out, np.float32).T] * 2))

    nc = _build_program(Ks, IDXC, WTC, use_wt=not allones)
    in_maps = [{
        "h_in": np.ascontiguousarray(hp[c]),
        "idx_in": idx_dev[c],
        "wt_in": wt_dev[c],
        "w0t_in": w0t, "w1t_in": w1t, "wot_in": wot,
        "pc_in": pc_dev[c],
    } for c in range(C)]
    res = run_bass_kernel_spmd(nc, in_maps, list(range(C)), trace=_trace)
    od = np.stack([res.results[c]["out"] for c in range(C)])
    od = od.reshape(C, P, T, 16).transpose(0, 2, 1, 3).reshape(N, 16)
    out = np.empty((N, 16), np.float32)
    out[perm] = od
    if _trace:
        return out, res
    return out
